# revision 63
# baseline (speedup 1.0000x reference)
"""LoRA-MoE layer (base dense + top-2 routed rank-16 LoRA experts) on 8 TRN2 cores.

Data-parallel over tokens (8192 -> 1024/core), weights replicated, zero
collectives.  The dense base projection runs as fp8e4 DoubleRow matmuls
(K=256 packed per instruction, 2 output columns/cycle) with a hi/lo split:
    x@W ~= xh@Wh + xh@Wl + xl@Wh      xh = fp8(x), xl = fp8(x - xh)
W/A/R are pre-scaled on host (x512/x32/x32) so their uniform(+-1/sqrt(2048))
values escape fp8's subnormal range; the W-scale rides through PSUM (Bc
carries it too) and is divided out on the host after gather.  The xl
correction is dropped for kp>=4 (half the contraction): measured full-batch
error 1.755e-2 against the 2e-2 gate, for 25% fewer base matmuls.

PE-sequencer economics dominate instruction choice: every Ldweights costs
~105ns of PE SEQ while a matmult costs ~2ns, so matmuls are grouped
(ti, ob-pair)-major so runs of 4-6 share one stationary, and a post-pass
(_dedupe_ldweights) deletes the redundant weight loads the tile scheduler
emits 1:1.

Schedule: 8 k-pair chunks; PSUM holds only ~6 rotating accumulators, so
sweeps S0 (kp0), S1 (kp1), S2 (kp2+3) evacuate per-tile partials into an
SBUF f32 accumulator (ACT/DVE copies + DVE adds - GPSIMD cannot touch PSUM
on real HW; a minority of adds go ACT-copy + Pool SBUF-add), and S3 (kp4-7)
fuses the fp8-DR LoRA-B finisher and a staging add before each store.  Token
tile 7 skips the early sweeps entirely and runs whole-K per-ob groups at the
tail, storing each ob as soon as it finishes.  The DMA stream is ordered so
every sweep's W chunks land just ahead of their consumers (W0 halves split
finer for the prologue, W1 behind x1, W2/W3 braided into the x tail, W4-7
last); merged hi/lo tensors keep the 625ns-per-descriptor HWDGE issue rate
off the critical path.

Routing: per (ti, kp) three ap-8 DoubleRow matmuls (xh.Rh + xh.Rl + xl.Rh)
ride the base x stationaries, accumulating token-major [128t, 8e] regions of
one PSUM bank as sequential per-ti bursts; the top-2 softmax chains (DVE/ACT)
consume them directly - no logit transposes.  u = A.x is 1-term fp8 DR.
Per-token weights are transposed and expanded to [er, t] via the one-hot Mm
matmul; us = u * wb is then re-packed to the DoubleRow [64, 2, t] layout with
PE selector matmuls (DVE cannot shift partitions) so the finisher runs fp8.
"""

import os
import sys

import numpy as np


def _ensure_concourse():
    try:
        import concourse  # noqa: F401
    except ImportError:
        for p in ("/opt/trn_rl_repo", os.path.expanduser("~/.axon_site/_ro/trn_rl_repo")):
            if os.path.isdir(p):
                sys.path.insert(0, p)
                break


_ensure_concourse()

import ml_dtypes  # noqa: E402
import concourse.bass as bass  # noqa: E402,F401
import concourse.tile as tile  # noqa: E402
from concourse import bacc, mybir  # noqa: E402

F32 = mybir.dt.float32
BF16 = mybir.dt.bfloat16
F8 = mybir.dt.float8e4
X_AX = mybir.AxisListType.X
ALU = mybir.AluOpType
ACT = mybir.ActivationFunctionType
DR = mybir.MatmulPerfMode.DoubleRow

N_CORES = 8
N_TOK = 8192          # total tokens (4 x 2048)
NT = N_TOK // N_CORES  # tokens per core = 1024
D = 2048
O = 2048
E = 8
R = 16
ER = E * R            # 128
KP = D // 256         # 8 k-pair chunks (256 contraction each, DoubleRow-packed)
TI = NT // 128        # 8 token tiles
OBS = 4               # o blocks of 512

S_W = 512.0           # host scale on W (and Bc); divided out on host
S_A = 32.0            # host scale on lora_A; divided out at u evac
S_R = 32.0            # host scale on router_w; divided out at chain head

_NC_CACHE = {}
LAST_RESULTS = None


def _emit_chain(nc, smallp, lg_reg, w_tiles):
    """Top-2 softmax weight chain for one 128-token tile (DVE/ACT ops).

    lg_reg is the token-major [128, 8] f32 PSUM region holding S_R * logits."""
    L = smallp.tile([128, E], F32, name="L", tag="L")
    nc.scalar.mul(L[:], lg_reg, 1.0 / S_R)
    m1 = smallp.tile([128, 1], F32, name="m1", tag="m1")
    nc.vector.reduce_max(m1[:], L[:], axis=X_AX)
    nm1 = smallp.tile([128, 1], F32, name="nm1", tag="nm1")
    nc.scalar.mul(nm1[:], m1[:], -1.0)
    # mask out the top-1 entry, then find the 2nd max
    msk = smallp.tile([128, E], F32, name="msk", tag="msk")
    nc.vector.tensor_scalar(msk[:], L[:], m1[:], -1e30, ALU.is_equal, ALU.mult)
    L2 = smallp.tile([128, E], F32, name="L2", tag="L2")
    nc.vector.tensor_tensor(L2[:], L[:], msk[:], ALU.add)
    m2 = smallp.tile([128, 1], F32, name="m2", tag="m2")
    nc.vector.reduce_max(m2[:], L2[:], axis=X_AX)
    eL = smallp.tile([128, E], F32, name="eL", tag="eL")
    nc.scalar.activation(eL[:], L[:], ACT.Exp, bias=nm1[:])
    ge = smallp.tile([128, E], F32, name="ge", tag="ge")
    nc.vector.tensor_scalar(ge[:], L[:], m2[:], None, ALU.is_ge)
    un = smallp.tile([128, E], F32, name="un", tag="un")
    nc.vector.tensor_tensor(un[:], eL[:], ge[:], ALU.mult)
    s = smallp.tile([128, 1], F32, name="s", tag="s")
    nc.vector.reduce_sum(s[:], un[:], axis=X_AX)
    r = smallp.tile([128, 1], F32, name="r", tag="r")
    nc.vector.reciprocal(r[:], s[:])
    r2 = smallp.tile([128, 1], F32, name="r2", tag="r2")
    nc.scalar.mul(r2[:], r[:], 2.0)  # fold SCALING = 2.0
    w = smallp.tile([128, E], BF16, name="w", tag="w", bufs=8)
    nc.vector.tensor_scalar(w[:], un[:], r2[:], None, ALU.mult)
    w_tiles.append(w)


def _body(tc, nc, xp, Wp, Ah, Rr, Bc, Sel, Mm, Idb, out):
    with (
        tc.tile_pool(name="const", bufs=1) as constp,
        tc.tile_pool(name="small", bufs=4) as smallp,
        tc.tile_pool(name="stage", bufs=4) as stagep,
    ):
        # PSUM: u(2) + lg(1) + rotation mm0..3 (4) + trwb (1) = 8 banks.
        ps_u = tc.alloc_tile_pool(name="ps_u", bufs=1, space="PSUM")
        ps_lg = tc.alloc_tile_pool(name="ps_lg", bufs=1, space="PSUM")
        ps_tr = tc.alloc_tile_pool(name="ps_tr", bufs=1, space="PSUM")
        ps_mm = tc.alloc_tile_pool(name="ps_mm", bufs=1, space="PSUM")

        # ---- resident SBUF tensors ----
        xp_sb = constp.tile([128, KP, 2, 2 * NT], F8, name="xp_sb")
        Wp_sb = constp.tile([128, KP, 2, 2 * O], F8, name="Wp_sb")
        Ah_sb = constp.tile([128, KP, 2, ER], F8, name="Ah_sb")
        Rr_sb = constp.tile([128, KP, 2, 32], F8, name="Rr_sb")
        Bc_sb = constp.tile([64, 2, O], F8, name="Bc_sb")
        Sel_sb = constp.tile([ER, ER], BF16, name="Sel_sb")
        us8_sb = constp.tile([64, 2, NT], F8, name="us8_sb")
        Mm_sb = constp.tile([E, ER], BF16, name="Mm_sb")
        Idb_sb = constp.tile([128, 128], BF16, name="Idb_sb")
        u_sb = constp.tile([ER, NT], F32, name="u_sb")
        us_sb = constp.tile([ER, NT], BF16, name="us_sb")
        wT_sb = constp.tile([E, NT], BF16, name="wT_sb")
        part_sb = constp.tile([128, 32 * 512], F32, name="part_sb")

        tiles = [(ti, ob) for ti in range(TI) for ob in range(OBS)]  # 32

        # PE p-state warm-up: the cost model runs PE at 0.65-1.2GHz for the
        # first 3us of busy time.  Burn the ramp on throwaway fp32 matmuls
        # (iota-seeded, no DMA dependency) while the first chunks stream in.
        # PE p-state warm-up: the model halves PE speed for the first 3us
        # of busy time; burn most of the ramp on throwaway matmuls while the
        # first DMA chunks land (out partitions = wu free dim = 8).
        wu_sb = constp.tile([128, 520], F32, name="wu_sb")
        wu_ps = ps_lg.tile([8, 512], F32, name="wu_ps", tag="lg",
                           padded_shape=[128, 512])
        nc.gpsimd.memset(wu_sb[:], 1.0)
        nc.tensor.matmul(wu_ps[:], wu_sb[:, 0:8], wu_sb[:, 8:520],
                         start=True, stop=True)

        # ---- DMA emission (single in-order SP/HWDGE stream) ----
        # One dma_start per merged hi/lo chunk (the 625ns HWDGE issue cost
        # otherwise rate-limits).  First x/W chunks split finer so PE starts
        # ~3us in; W kp1 behind x kp1 (S0/S1 never W-gated); W2 early, then
        # the x tail, W3 just ahead of S2's kp3 rows, Bc, and the W tail
        # ahead of S3.
        # first chunks issue from the ACT queue in parallel with SP's so
        # both DGE pipelines fill while the DMA engines are still empty
        nc.sync.dma_start(Ah_sb[:], Ah[:])
        nc.sync.dma_start(xp_sb[:, 0, :, 0:512], xp[:, 0, :, 0:512])
        nc.sync.dma_start(xp_sb[:, 0, :, 512:1024], xp[:, 0, :, 512:1024])
        nc.scalar.dma_start(Wp_sb[:, 0, :, 0:1024], Wp[:, 0, :, 0:1024])
        nc.sync.dma_start(xp_sb[:, 0, :, 1024:2048], xp[:, 0, :, 1024:2048])
        nc.sync.dma_start(Wp_sb[:, 0, :, 2048:3072], Wp[:, 0, :, 2048:3072])
        nc.sync.dma_start(Rr_sb[:], Rr[:])
        nc.sync.dma_start(Wp_sb[:, 0, :, 1024:2048], Wp[:, 0, :, 1024:2048])
        nc.sync.dma_start(Wp_sb[:, 0, :, 3072:4096], Wp[:, 0, :, 3072:4096])
        nc.sync.dma_start(Mm_sb[:], Mm[:])
        nc.sync.dma_start(Idb_sb[:], Idb[:])
        nc.sync.dma_start(xp_sb[:, 1], xp[:, 1])
        nc.sync.dma_start(Wp_sb[:, 1], Wp[:, 1])
        # x hi-halves first (base sweeps + u read only xh); the lo (xl)
        # halves of kp>=4 feed nothing but the S2 logit bursts, so they ship
        # after W3 - pulling every W chunk ~3us earlier
        for kp in range(2, 6):
            nc.sync.dma_start(xp_sb[:, kp, :, 0:NT], xp[:, kp, :, 0:NT])
        nc.sync.dma_start(xp_sb[:, 6, :, 0:NT], xp[:, 6, :, 0:NT])
        nc.sync.dma_start(Wp_sb[:, 2, :, 0:O], Wp[:, 2, :, 0:O])
        nc.sync.dma_start(xp_sb[:, 7, :, 0:NT], xp[:, 7, :, 0:NT])
        nc.sync.dma_start(Wp_sb[:, 2, :, O:2 * O], Wp[:, 2, :, O:2 * O])
        nc.sync.dma_start(Wp_sb[:, 3, :, 0:O], Wp[:, 3, :, 0:O])
        nc.sync.dma_start(xp_sb[:, 2, :, NT:2 * NT], xp[:, 2, :, NT:2 * NT])
        nc.sync.dma_start(xp_sb[:, 3, :, NT:2 * NT], xp[:, 3, :, NT:2 * NT])
        nc.sync.dma_start(Wp_sb[:, 3, :, O:2 * O], Wp[:, 3, :, O:2 * O])
        for kp in range(4, KP):
            nc.sync.dma_start(xp_sb[:, kp, :, NT:2 * NT],
                              xp[:, kp, :, NT:2 * NT])
        nc.sync.dma_start(Bc_sb[:], Bc[:])
        nc.sync.dma_start(Sel_sb[:], Sel[:])
        for kp in range(4, KP):
            nc.sync.dma_start(Wp_sb[:, kp], Wp[:, kp])

        # ---- u / lg accumulators ----
        u_ps = [ps_u.tile([ER, 512], F32, name=f"ups{tb}", tag=f"u{tb}")
                for tb in range(2)]
        lg_ps = None  # created at S2 (the lg bank joins the S0/S1 rotation)

        def xs(which, kp, sl):
            off = 0 if which == "h" else NT
            return xp_sb[:, kp, :, off + sl.start:off + sl.stop]

        def emit_ulg(kp):
            st, sp = (kp == 0), (kp == KP - 1)
            for tb in range(2):
                nc.tensor.matmul(u_ps[tb][:], Ah_sb[:, kp],
                                 xs("h", kp, slice(tb * 512, (tb + 1) * 512)),
                                 start=st, stop=sp, perf_mode=DR)

        lg_holder = []

        def emit_lg_burst(ti):
            # one sequential accumulation group per [128t, 8e] region (the
            # interp allows a single pending group per psum tile): all 8 kp
            # x 3 hi/lo terms back-to-back, then the chain consumes it.
            reg = lg_holder[0][:, ti * E:(ti + 1) * E]
            tsl = slice(ti * 128, (ti + 1) * 128)
            n = KP * 3
            i = 0
            for kp in range(KP):
                for which, roff in (("h", 0), ("h", 16), ("l", 0)):
                    nc.tensor.matmul(reg, xs(which, kp, tsl),
                                     Rr_sb[:, kp, :, roff:roff + E],
                                     start=(i == 0), stop=(i == n - 1),
                                     perf_mode=DR)
                    i += 1
            _emit_chain(nc, smallp, reg, w_tiles)

        rot = [0]
        TAGS5 = [(ps_mm, "mm0"), (ps_mm, "mm1"), (ps_mm, "mm2"),
                 (ps_mm, "mm3"), (ps_tr, "trwb"), (ps_lg, "lg")]
        TAGS6 = [(ps_mm, "mm0"), (ps_mm, "mm1"), (ps_mm, "mm2"),
                 (ps_mm, "mm3"), (ps_u, "u0"), (ps_u, "u1"),
                 (ps_tr, "trwb")]
        TAGS7 = TAGS6 + [(ps_lg, "lg")]
        TAGS8 = TAGS7 + [(ps_tr, "trwb")]

        def rot_tile(name, tags):
            pool, tag = tags[rot[0] % len(tags)]
            rot[0] += 1
            return pool.tile([128, 512], F32, name=name, tag=tag, bufs=1)

        def emit_pair_group(ti, p, kps, tags, sweep, last_sweep=False):
            """One (token-tile, ob-pair) group: per kp, 4 xh-stationary
            matmuls (Wh/Wl x 2 ob) then 2 xl-stationary (Wh x 2 ob); the
            ldweights dedupe collapses each run to one weight load.  Returns
            the 2 psum tiles (ob 2p, 2p+1)."""
            tsl = slice(ti * 128, (ti + 1) * 128)
            pss = [rot_tile(f"{sweep}_{ti}_{2 * p + i}", tags)
                   for i in range(2)]
            first_kp, last_kp = kps[0], kps[-1]
            for kp in kps:
                for i in range(2):
                    ob = 2 * p + i
                    osl = slice(ob * 512, (ob + 1) * 512)
                    nc.tensor.matmul(pss[i][:], xs("h", kp, tsl),
                                     Wp_sb[:, kp, :, osl.start:osl.stop],
                                     start=(kp == first_kp), stop=False,
                                     perf_mode=DR)
                    nc.tensor.matmul(pss[i][:], xs("h", kp, tsl),
                                     Wp_sb[:, kp, :, O + osl.start:O + osl.stop],
                                     start=False, stop=False, perf_mode=DR)
                for i in range(2):
                    ob = 2 * p + i
                    osl = slice(ob * 512, (ob + 1) * 512)
                    # the xl correction is dropped for kp 6-7: the remaining
                    # x-quantization noise on 2/8 of the contraction is
                    # ~1.3% of the output against the 2e-2 gate, and it saves
                    # 2 matmuls per (pair, kp) plus the xl bytes of the
                    # stream tail
                    if kp >= 4:
                        continue
                    nc.tensor.matmul(pss[i][:], xs("l", kp, tsl),
                                     Wp_sb[:, kp, :, osl.start:osl.stop],
                                     start=False,
                                     stop=(kp == last_kp and not last_sweep),
                                     perf_mode=DR)
            return pss

        ev = [0]
        EV_COPY = ("act", "vec", "pool", "vec")
        EV_ADD = ("vec", "pool", "vec", "vec")

        def emit_evac(ps, idx, first):
            # kp0 sweep: copy psum -> f32 partial; later sweeps: partial +=
            # psum.  ACT only ever sees copies (no tensor_tensor on the
            # scalar engine); Pool's f32 tensor ops are ~2.3x slower than
            # DVE so it takes a minority share.
            # GPSIMD/Pool cannot access PSUM on real HW (BIR verifier), so
            # copies alternate ACT/DVE and adds are DVE-only.
            dst = part_sb[:, idx * 512:(idx + 1) * 512]
            e = (EV_COPY if first else EV_ADD)[ev[0] % 4]
            ev[0] += 1
            if first:
                if e in ("act", "pool"):
                    nc.scalar.copy(dst, ps[:])
                else:
                    nc.vector.tensor_copy(dst, ps[:])
            elif e == "pool":
                # relieve DVE: ACT evacuates PSUM to a scratch tile, Pool
                # (SBUF-only) folds it into the partial
                sc = stagep.tile([128, 512], F32, name="sc", tag="sc", bufs=2)
                nc.scalar.copy(sc[:], ps[:])
                nc.gpsimd.tensor_tensor(dst, dst, sc[:], ALU.add)
            else:
                nc.vector.tensor_tensor(dst, dst, ps[:], ALU.add)

        w_tiles = []

        def emit_wexpand(tb):
            for ti in range(tb * 4, tb * 4 + 4):
                sl = slice(ti * 128, (ti + 1) * 128)
                trW = ps_tr.tile([E, 128], BF16, name="trW", tag="trwb",
                                 padded_shape=[128, 1024])
                nc.tensor.transpose(trW[:], w_tiles[ti][:], Idb_sb[:])
                nc.scalar.copy(wT_sb[:, sl], trW[:])
            sl = slice(tb * 512, (tb + 1) * 512)
            wb_ps = ps_tr.tile([ER, 512], F32, name="wbps", tag="trwb")
            nc.tensor.matmul(wb_ps[:], Mm_sb[:], wT_sb[:, sl],
                             start=True, stop=True)
            nc.vector.tensor_tensor(us_sb[:, sl], u_sb[:, sl],
                                    wb_ps[:], ALU.mult)
            # pack us rows into the DoubleRow [64, 2, t] layout for the fp8
            # finisher: PE selector matmuls move er 64..127 onto partitions
            # 0..63 (DVE cannot shift partitions), ACT casts psum -> fp8
            for j in range(2):
                pk = ps_tr.tile([64, 512], F32, name="pk", tag="trwb",
                                padded_shape=[128, 512])
                nc.tensor.matmul(pk[:], Sel_sb[:, j * 64:(j + 1) * 64],
                                 us_sb[:, sl], start=True, stop=True)
                if j == 0:
                    nc.scalar.copy(us8_sb[:, j, sl.start:sl.stop], pk[:])
                else:
                    nc.vector.tensor_copy(us8_sb[:, j, sl.start:sl.stop],
                                          pk[:])

        def emit_finish(pss4, ti, copy_stage=False):
            # fused bf16 LoRA-B finishers for all 4 ob tiles of this token
            # tile (one shared us stationary), then staging (+ partial when
            # the tile ran the early sweeps) and the store.
            tsl = slice(ti * 128, (ti + 1) * 128)
            for ob in range(OBS):
                nc.tensor.matmul(pss4[ob][:], us8_sb[:, :, tsl.start:tsl.stop],
                                 Bc_sb[:, :, ob * 512:(ob + 1) * 512],
                                 start=False, stop=True, perf_mode=DR)
            for ob in range(OBS):
                idx = ti * OBS + ob
                st = stagep.tile([128, 512], BF16, name="st", tag="st", bufs=6)
                if copy_stage:
                    if ob % 2 == 0:
                        nc.scalar.copy(st[:], pss4[ob][:])
                    else:
                        nc.vector.tensor_copy(st[:], pss4[ob][:])
                else:
                    nc.vector.tensor_tensor(st[:], pss4[ob][:],
                                            part_sb[:, idx * 512:(idx + 1) * 512],
                                            ALU.add)
                nc.sync.dma_start(out[tsl, ob * 512:(ob + 1) * 512], st[:])

        NTI = TI - 1  # ti7 skips the early sweeps: full sweep at the tail

        # ---- S0: kp0 sweep (ti 0..6); u(0) up front, u(1..2) as x lands.
        # p-major order: all ob-pair-0 groups need only the first halves of
        # the W0 stream, so PE stops chasing the W0 DMA after ~2 chunks. ----
        emit_ulg(0)

        def s0_hi(ti, pss):
            # all rows reading the Wh half of W0 (xh.Wh and xl.Wh) so the
            # prologue isn't gated on the later Wl sub-chunk
            tsl = slice(ti * 128, (ti + 1) * 128)
            for which in ("h", "l"):
                for i in range(2):
                    osl = slice(i * 512, (i + 1) * 512)
                    nc.tensor.matmul(pss[i][:], xs(which, 0, tsl),
                                     Wp_sb[:, 0, :, osl.start:osl.stop],
                                     start=(which == "h"), stop=False,
                                     perf_mode=DR)

        def s0_lo(ti, pss):
            tsl = slice(ti * 128, (ti + 1) * 128)
            for i in range(2):
                osl = slice(i * 512, (i + 1) * 512)
                nc.tensor.matmul(pss[i][:], xs("h", 0, tsl),
                                 Wp_sb[:, 0, :, O + osl.start:O + osl.stop],
                                 start=False, stop=True, perf_mode=DR)

        g0 = [rot_tile("s0e_0_%d" % i, TAGS5) for i in range(2)]
        g1 = [rot_tile("s0e_1_%d" % i, TAGS5) for i in range(2)]
        s0_hi(0, g0)
        s0_hi(1, g1)
        s0_lo(0, g0)
        for i in range(2):
            emit_evac(g0[i], 0 * OBS + i, first=True)
        g2 = [rot_tile("s0e_2_%d" % i, TAGS5) for i in range(2)]
        s0_hi(2, g2)
        s0_lo(1, g1)
        for i in range(2):
            emit_evac(g1[i], 1 * OBS + i, first=True)
        s0_lo(2, g2)
        for i in range(2):
            emit_evac(g2[i], 2 * OBS + i, first=True)
        for p in range(2):
            for ti in range(3 if p == 0 else 0, NTI):
                if (ti, p) == (0, 1):
                    emit_ulg(1)
                if (ti, p) == (4, 1):
                    emit_ulg(2)
                pss = emit_pair_group(ti, p, [0], TAGS5, "s0")
                for i in range(2):
                    emit_evac(pss[i], ti * OBS + 2 * p + i, first=True)

        # ---- S1: kp1 sweep; u(3..7) braided by x arrival; u evac ----
        emit_ulg(3)
        for ti in range(NTI):
            for p in range(2):
                if (ti, p) == (1, 1):
                    emit_ulg(4)
                if (ti, p) == (3, 1):
                    emit_ulg(5)
                if (ti, p) == (5, 1):
                    emit_ulg(6)
                pss = emit_pair_group(ti, p, [1], TAGS5, "s1")
                for i in range(2):
                    emit_evac(pss[i], ti * OBS + 2 * p + i, first=False)
        emit_ulg(7)
        for tb in range(2):
            nc.scalar.mul(u_sb[:, tb * 512:(tb + 1) * 512], u_ps[tb][:],
                          1.0 / S_A)

        # ---- S2: kp2+3 paired sweep; lg bursts + chains; w expansions ----
        lg_holder.append(ps_lg.tile([128, 8 * E], F32, name="lg", tag="lg",
                                    padded_shape=[128, 512]))
        for ti in range(NTI):
            for p in range(2):
                if ti in (2, 3):
                    b = 4 * (ti - 2) + 2 * p
                    emit_lg_burst(b)
                    emit_lg_burst(b + 1)
                if (ti, p) == (4, 1):
                    emit_wexpand(0)
                if (ti, p) == (6, 0):
                    emit_wexpand(1)
                pss = emit_pair_group(ti, p, [2, 3], TAGS6, "s2")
                for i in range(2):
                    emit_evac(pss[i], ti * OBS + 2 * p + i, first=False)

        # ---- S3: ti7's first two whole-K ob groups lead (their kp0-3 rows
        # are DMA-independent, absorbing the W tail window), then the per-ti
        # kp4..7 sweeps + finishers, then ti7's deferred finishers and its
        # last two obs.  The last stores issue from the ACT/DVE queues so
        # their HWDGE work runs off the SP path in the kernel tail. ----
        tsl7 = slice(7 * 128, 8 * 128)

        def ti7_base(ob, ps):
            osl = slice(ob * 512, (ob + 1) * 512)
            for kp in range(KP):
                nc.tensor.matmul(ps[:], xs("h", kp, tsl7),
                                 Wp_sb[:, kp, :, osl.start:osl.stop],
                                 start=(kp == 0), stop=False, perf_mode=DR)
                nc.tensor.matmul(ps[:], xs("h", kp, tsl7),
                                 Wp_sb[:, kp, :, O + osl.start:O + osl.stop],
                                 start=False, stop=False, perf_mode=DR)
                if kp < 4:
                    nc.tensor.matmul(ps[:], xs("l", kp, tsl7),
                                     Wp_sb[:, kp, :, osl.start:osl.stop],
                                     start=False, stop=False, perf_mode=DR)

        def ti7_fin(ob, ps, eng):
            osl = slice(ob * 512, (ob + 1) * 512)
            nc.tensor.matmul(ps[:], us8_sb[:, :, tsl7.start:tsl7.stop],
                             Bc_sb[:, :, osl.start:osl.stop],
                             start=False, stop=True, perf_mode=DR)
            st = stagep.tile([128, 512], BF16, name="st", tag="st", bufs=6)
            if eng == "act":
                nc.scalar.copy(st[:], ps[:])
            else:
                nc.vector.tensor_copy(st[:], ps[:])
            nc.sync.dma_start(out[tsl7, osl], st[:])

        TAGS6C = [(ps_mm, "mm0"), (ps_mm, "mm1"), (ps_mm, "mm2"),
                  (ps_mm, "mm3"), (ps_lg, "lg"), (ps_tr, "trwb")]
        # ti7 ob0/ob1 park on the freed u banks (outside the rotation) so
        # their finishers can wait for us8 without blocking the loop
        t7a = ps_u.tile([128, 512], F32, name="s3f_0", tag="u0", bufs=1)
        ti7_base(0, t7a)
        t7b = ps_u.tile([128, 512], F32, name="s3f_1", tag="u1", bufs=1)
        ti7_base(1, t7b)
        for ti in range(NTI):
            pssA = emit_pair_group(ti, 0, [4, 5, 6, 7], TAGS6C, "s3",
                                   last_sweep=True)
            pssB = emit_pair_group(ti, 1, [4, 5, 6, 7], TAGS6C, "s3",
                                   last_sweep=True)
            emit_finish(pssA + pssB, ti)
        ti7_fin(0, t7a, "act")
        ti7_fin(1, t7b, "vec")
        t7c = rot_tile("s3f_2", TAGS6C)
        ti7_base(2, t7c)
        ti7_fin(2, t7c, "act")
        t7d = rot_tile("s3f_3", TAGS6C)
        ti7_base(3, t7d)
        ti7_fin(3, t7d, "vec")

        ps_mm.release()
        ps_tr.release()
        ps_lg.release()
        ps_u.release()


def _ldweights_key(inst):
    ap = inst.ins[0]
    return (str(ap), str(inst.perf_mode), str(inst.is_transpose),
            str(inst.tile_position), str(inst.tile_size))


def _dedupe_ldweights(nc):
    """Drop an InstLdweights when the PE array already holds the same
    stationary (identical weights AP, only paired matmults in between).
    The ~105ns-per-instruction PE sequencer cost of redundant weight loads
    otherwise dominates the kernel."""
    removed = 0
    for bb in nc.m.functions[0].blocks:
        keep = []
        last_key = None
        for inst in bb.instructions:
            t = type(inst).__name__
            if t == "InstLdweights":
                k = _ldweights_key(inst)
                si = inst.sync_info
                has_sync = si is not None and (list(si.on_wait) or
                                               list(si.on_update))
                if k == last_key and not has_sync:
                    removed += 1
                    continue
                last_key = k
            elif t != "InstMatmult":
                if getattr(inst, "engine", None) == mybir.EngineType.PE:
                    last_key = None
            keep.append(inst)
        bb.instructions = keep
    return removed


def build_nc():
    nc = bacc.Bacc("TRN2", target_bir_lowering=False, debug=False, num_devices=N_CORES)
    xp = nc.dram_tensor("xp", [128, KP, 2, 2 * NT], F8, kind="ExternalInput").ap()
    Wp = nc.dram_tensor("Wp", [128, KP, 2, 2 * O], F8, kind="ExternalInput").ap()
    Ah = nc.dram_tensor("Ah", [128, KP, 2, ER], F8, kind="ExternalInput").ap()
    Rr = nc.dram_tensor("Rr", [128, KP, 2, 32], F8, kind="ExternalInput").ap()
    Bc = nc.dram_tensor("Bc", [64, 2, O], F8, kind="ExternalInput").ap()
    Sel = nc.dram_tensor("Sel", [ER, ER], BF16, kind="ExternalInput").ap()
    Mm = nc.dram_tensor("Mm", [E, ER], BF16, kind="ExternalInput").ap()
    Idb = nc.dram_tensor("Idb", [128, 128], BF16, kind="ExternalInput").ap()
    out = nc.dram_tensor("out", [NT, O], BF16, kind="ExternalOutput").ap()
    with tile.TileContext(nc) as tc:
        _body(tc, nc, xp, Wp, Ah, Rr, Bc, Sel, Mm, Idb, out)
    _dedupe_ldweights(nc)
    nc.compile()
    return nc


def get_nc():
    if "nc" not in _NC_CACHE:
        _NC_CACHE["nc"] = build_nc()
    return _NC_CACHE["nc"]


F8NP = ml_dtypes.float8_e4m3


def _pack_k(aT):
    """[D, C] -> [128, KP, 2, C]: element [p, kp, j, :] holds row k=kp*256+j*128+p."""
    C = aT.shape[1]
    return np.ascontiguousarray(
        aT.reshape(KP, 2, 128, C).transpose(2, 0, 1, 3))


def _hi_lo(aT):
    hi = aT.astype(F8NP)
    lo = (aT - hi.astype(np.float32)).astype(F8NP)
    return hi, lo


def make_in_maps(x, weight, lora_A, lora_B, router_w):
    x = np.ascontiguousarray(np.asarray(x, dtype=np.float32)).reshape(N_TOK, D)
    weight = np.asarray(weight, dtype=np.float32)
    lora_A = np.asarray(lora_A, dtype=np.float32)
    lora_B = np.asarray(lora_B, dtype=np.float32)
    router_w = np.asarray(router_w, dtype=np.float32)

    WTh, WTl = _hi_lo(np.ascontiguousarray(weight.T) * S_W)
    Wpm = np.concatenate([_pack_k(WTh), _pack_k(WTl)], axis=3)
    ATh = (np.ascontiguousarray(lora_A.reshape(ER, D).T) * S_A).astype(F8NP)
    Ahm = _pack_k(ATh)
    RT = np.zeros((D, 16), dtype=np.float32)
    RT[:, :E] = router_w.T * S_R
    RTh, RTl = _hi_lo(RT)
    Rrm = np.concatenate([_pack_k(RTh), _pack_k(RTl)], axis=3)
    BcT = lora_B.transpose(0, 2, 1).reshape(ER, O) * S_W
    # DR-packed: Bc8[p, j, o] = BcT[j*64 + p, o]
    Bcm = np.ascontiguousarray(BcT.reshape(2, 64, O).transpose(1, 0, 2)).astype(F8NP)
    Selm = np.zeros((ER, ER), dtype=np.float32)
    for j in range(2):
        for m in range(64):
            Selm[j * 64 + m, j * 64 + m] = 1.0
    # lhsT selector: out[m,t] = sum_er Sel[er, j*64+m-block] us[er, t]
    Selm = Selm.astype(ml_dtypes.bfloat16)
    Mmm = np.zeros((E, ER), dtype=np.float32)
    for e in range(E):
        Mmm[e, e * R:(e + 1) * R] = 1.0
    Mmm = Mmm.astype(ml_dtypes.bfloat16)
    Idb = np.eye(128, dtype=np.float32).astype(ml_dtypes.bfloat16)

    in_maps = []
    for c in range(N_CORES):
        xT = np.ascontiguousarray(x[c * NT:(c + 1) * NT].T)
        xTh, xTl = _hi_lo(xT)
        in_maps.append({
            "xp": np.concatenate([_pack_k(xTh), _pack_k(xTl)], axis=3),
            "Wp": Wpm,
            "Ah": Ahm,
            "Rr": Rrm,
            "Bc": Bcm,
            "Sel": Selm,
            "Mm": Mmm,
            "Idb": Idb,
        })
    return in_maps


def kernel(x, weight, lora_A, lora_B, router_w):
    global LAST_RESULTS
    from concourse.bass_utils import run_bass_kernel_spmd

    in_maps = make_in_maps(x, weight, lora_A, lora_B, router_w)
    nc = get_nc()
    trace = bool(os.environ.get("KBENCH_TRACE"))
    res = run_bass_kernel_spmd(nc, in_maps, core_ids=list(range(N_CORES)), trace=trace)
    LAST_RESULTS = res
    outs = [np.asarray(res.results[c]["out"], dtype=np.float32) * (1.0 / S_W)
            for c in range(N_CORES)]
    return np.concatenate(outs, axis=0).reshape(4, 2048, 2048)


# revision 67
# speedup vs baseline: 1.0042x; 1.0042x over previous
"""LoRA-MoE layer (base dense + top-2 routed rank-16 LoRA experts) on 8 TRN2 cores.

Data-parallel over tokens (8192 -> 1024/core), weights replicated, zero
collectives.  The dense base projection runs as fp8e4 DoubleRow matmuls
(K=256 packed per instruction, 2 output columns/cycle) with a hi/lo split:
    x@W ~= xh@Wh + xh@Wl + xl@Wh      xh = fp8(x), xl = fp8(x - xh)
W/A/R are pre-scaled on host (x512/x32/x32) so their uniform(+-1/sqrt(2048))
values escape fp8's subnormal range; the W-scale rides through PSUM (Bc
carries it too) and is divided out on the host after gather.  The xl
correction is dropped for kp>=4 (half the contraction): measured full-batch
error 1.755e-2 against the 2e-2 gate, for 25% fewer base matmuls.

PE-sequencer economics dominate instruction choice: every Ldweights costs
~105ns of PE SEQ while a matmult costs ~2ns, so matmuls are grouped
(ti, ob-pair)-major so runs of 4-6 share one stationary, and a post-pass
(_dedupe_ldweights) deletes the redundant weight loads the tile scheduler
emits 1:1.

Schedule: 8 k-pair chunks; PSUM holds only ~6 rotating accumulators, so
sweeps S0 (kp0), S1 (kp1), S2 (kp2+3) evacuate per-tile partials into an
SBUF f32 accumulator (ACT/DVE copies + DVE adds - GPSIMD cannot touch PSUM
on real HW; a minority of adds go ACT-copy + Pool SBUF-add), and S3 (kp4-7)
fuses the fp8-DR LoRA-B finisher and a staging add before each store.  Token
tile 7 skips the early sweeps entirely and runs whole-K per-ob groups at the
tail, storing each ob as soon as it finishes.  The DMA stream is ordered so
every sweep's W chunks land just ahead of their consumers (W0 halves split
finer for the prologue, W1 behind x1, W2/W3 braided into the x tail, W4-7
last); merged hi/lo tensors keep the 625ns-per-descriptor HWDGE issue rate
off the critical path.

Routing: per (ti, kp) three ap-8 DoubleRow matmuls (xh.Rh + xh.Rl + xl.Rh)
ride the base x stationaries, accumulating token-major [128t, 8e] regions of
one PSUM bank as sequential per-ti bursts; the top-2 softmax chains (DVE/ACT)
consume them directly - no logit transposes.  u = A.x is 1-term fp8 DR.
Per-token weights are transposed and expanded to [er, t] via the one-hot Mm
matmul; us = u * wb is then re-packed to the DoubleRow [64, 2, t] layout with
PE selector matmuls (DVE cannot shift partitions) so the finisher runs fp8.
"""

import os
import sys

import numpy as np


def _ensure_concourse():
    try:
        import concourse  # noqa: F401
    except ImportError:
        for p in ("/opt/trn_rl_repo", os.path.expanduser("~/.axon_site/_ro/trn_rl_repo")):
            if os.path.isdir(p):
                sys.path.insert(0, p)
                break


_ensure_concourse()

import ml_dtypes  # noqa: E402
import concourse.bass as bass  # noqa: E402,F401
import concourse.tile as tile  # noqa: E402
from concourse import bacc, mybir  # noqa: E402

F32 = mybir.dt.float32
BF16 = mybir.dt.bfloat16
F8 = mybir.dt.float8e4
X_AX = mybir.AxisListType.X
ALU = mybir.AluOpType
ACT = mybir.ActivationFunctionType
DR = mybir.MatmulPerfMode.DoubleRow

N_CORES = 8
N_TOK = 8192          # total tokens (4 x 2048)
NT = N_TOK // N_CORES  # tokens per core = 1024
D = 2048
O = 2048
E = 8
R = 16
ER = E * R            # 128
KP = D // 256         # 8 k-pair chunks (256 contraction each, DoubleRow-packed)
TI = NT // 128        # 8 token tiles
OBS = 4               # o blocks of 512

S_W = 512.0           # host scale on W (and Bc); divided out on host
S_A = 32.0            # host scale on lora_A; divided out at u evac
S_R = 32.0            # host scale on router_w; divided out at chain head

_NC_CACHE = {}
LAST_RESULTS = None


def _emit_chain(nc, smallp, lg_reg, w_tiles):
    """Top-2 softmax weight chain for one 128-token tile (DVE/ACT ops).

    lg_reg is the token-major [128, 8] f32 PSUM region holding S_R * logits."""
    L = smallp.tile([128, E], F32, name="L", tag="L")
    nc.scalar.mul(L[:], lg_reg, 1.0 / S_R)
    m1 = smallp.tile([128, 1], F32, name="m1", tag="m1")
    nc.vector.reduce_max(m1[:], L[:], axis=X_AX)
    nm1 = smallp.tile([128, 1], F32, name="nm1", tag="nm1")
    nc.scalar.mul(nm1[:], m1[:], -1.0)
    # mask out the top-1 entry, then find the 2nd max
    msk = smallp.tile([128, E], F32, name="msk", tag="msk")
    nc.vector.tensor_scalar(msk[:], L[:], m1[:], -1e30, ALU.is_equal, ALU.mult)
    L2 = smallp.tile([128, E], F32, name="L2", tag="L2")
    nc.vector.tensor_tensor(L2[:], L[:], msk[:], ALU.add)
    m2 = smallp.tile([128, 1], F32, name="m2", tag="m2")
    nc.vector.reduce_max(m2[:], L2[:], axis=X_AX)
    eL = smallp.tile([128, E], F32, name="eL", tag="eL")
    nc.scalar.activation(eL[:], L[:], ACT.Exp, bias=nm1[:])
    ge = smallp.tile([128, E], F32, name="ge", tag="ge")
    nc.vector.tensor_scalar(ge[:], L[:], m2[:], None, ALU.is_ge)
    un = smallp.tile([128, E], F32, name="un", tag="un")
    nc.vector.tensor_tensor(un[:], eL[:], ge[:], ALU.mult)
    s = smallp.tile([128, 1], F32, name="s", tag="s")
    nc.vector.reduce_sum(s[:], un[:], axis=X_AX)
    r = smallp.tile([128, 1], F32, name="r", tag="r")
    nc.vector.reciprocal(r[:], s[:])
    r2 = smallp.tile([128, 1], F32, name="r2", tag="r2")
    nc.scalar.mul(r2[:], r[:], 2.0)  # fold SCALING = 2.0
    w = smallp.tile([128, E], BF16, name="w", tag="w", bufs=8)
    nc.vector.tensor_scalar(w[:], un[:], r2[:], None, ALU.mult)
    w_tiles.append(w)


def _body(tc, nc, xp, Wp, Ah, Rr, Bc, Sel, Mm, Idb, out):
    with (
        tc.tile_pool(name="const", bufs=1) as constp,
        tc.tile_pool(name="small", bufs=4) as smallp,
        tc.tile_pool(name="stage", bufs=4) as stagep,
    ):
        # PSUM: u(2) + lg(1) + rotation mm0..3 (4) + trwb (1) = 8 banks.
        ps_u = tc.alloc_tile_pool(name="ps_u", bufs=1, space="PSUM")
        ps_lg = tc.alloc_tile_pool(name="ps_lg", bufs=1, space="PSUM")
        ps_tr = tc.alloc_tile_pool(name="ps_tr", bufs=1, space="PSUM")
        ps_mm = tc.alloc_tile_pool(name="ps_mm", bufs=1, space="PSUM")

        # ---- resident SBUF tensors ----
        xp_sb = constp.tile([128, KP, 2, 2 * NT], F8, name="xp_sb")
        Wp_sb = constp.tile([128, KP, 2, 2 * O], F8, name="Wp_sb")
        Ah_sb = constp.tile([128, KP, 2, ER], F8, name="Ah_sb")
        Rr_sb = constp.tile([128, KP, 2, 32], F8, name="Rr_sb")
        Bc_sb = constp.tile([64, 2, O], F8, name="Bc_sb")
        Sel_sb = constp.tile([ER, ER], BF16, name="Sel_sb")
        us8_sb = constp.tile([64, 2, NT], F8, name="us8_sb")
        Mm_sb = constp.tile([E, ER], BF16, name="Mm_sb")
        Idb_sb = constp.tile([128, 128], BF16, name="Idb_sb")
        u_sb = constp.tile([ER, NT], F32, name="u_sb")
        us_sb = constp.tile([ER, NT], BF16, name="us_sb")
        wT_sb = constp.tile([E, NT], BF16, name="wT_sb")
        part_sb = constp.tile([128, 32 * 512], F32, name="part_sb")

        tiles = [(ti, ob) for ti in range(TI) for ob in range(OBS)]  # 32

        # PE p-state warm-up: the cost model runs PE at 0.65-1.2GHz for the
        # first 3us of busy time.  Burn the ramp on throwaway fp32 matmuls
        # (iota-seeded, no DMA dependency) while the first chunks stream in.
        # PE p-state warm-up: the model halves PE speed for the first 3us
        # of busy time; burn most of the ramp on throwaway matmuls while the
        # first DMA chunks land (out partitions = wu free dim = 8).
        wu_sb = constp.tile([128, 520], F32, name="wu_sb")
        wu_ps = ps_lg.tile([8, 512], F32, name="wu_ps", tag="lg",
                           padded_shape=[128, 512])
        nc.gpsimd.memset(wu_sb[:], 1.0)
        nc.tensor.matmul(wu_ps[:], wu_sb[:, 0:8], wu_sb[:, 8:520],
                         start=True, stop=True)

        # ---- DMA emission (single in-order SP/HWDGE stream) ----
        # One dma_start per merged hi/lo chunk (the 625ns HWDGE issue cost
        # otherwise rate-limits).  First x/W chunks split finer so PE starts
        # ~3us in; W kp1 behind x kp1 (S0/S1 never W-gated); W2 early, then
        # the x tail, W3 just ahead of S2's kp3 rows, Bc, and the W tail
        # ahead of S3.
        # first chunks issue from the ACT queue in parallel with SP's so
        # both DGE pipelines fill while the DMA engines are still empty
        nc.sync.dma_start(Ah_sb[:], Ah[:])
        nc.sync.dma_start(xp_sb[:, 0, :, 0:512], xp[:, 0, :, 0:512])
        nc.sync.dma_start(xp_sb[:, 0, :, 512:1024], xp[:, 0, :, 512:1024])
        nc.scalar.dma_start(Wp_sb[:, 0, :, 0:1024], Wp[:, 0, :, 0:1024])
        nc.sync.dma_start(xp_sb[:, 0, :, 1024:2048], xp[:, 0, :, 1024:2048])
        nc.sync.dma_start(Wp_sb[:, 0, :, 2048:3072], Wp[:, 0, :, 2048:3072])
        nc.sync.dma_start(Rr_sb[:], Rr[:])
        nc.sync.dma_start(Wp_sb[:, 0, :, 1024:2048], Wp[:, 0, :, 1024:2048])
        nc.sync.dma_start(Wp_sb[:, 0, :, 3072:4096], Wp[:, 0, :, 3072:4096])
        nc.sync.dma_start(Mm_sb[:], Mm[:])
        nc.sync.dma_start(Idb_sb[:], Idb[:])
        nc.sync.dma_start(xp_sb[:, 1], xp[:, 1])
        nc.sync.dma_start(Wp_sb[:, 1], Wp[:, 1])
        # x hi-halves first (base sweeps + u read only xh); the lo (xl)
        # halves of kp>=4 feed nothing but the S2 logit bursts, so they ship
        # after W3 - pulling every W chunk ~3us earlier
        for kp in range(2, 6):
            nc.sync.dma_start(xp_sb[:, kp, :, 0:NT], xp[:, kp, :, 0:NT])
        nc.sync.dma_start(xp_sb[:, 6, :, 0:NT], xp[:, 6, :, 0:NT])
        nc.sync.dma_start(Wp_sb[:, 2, :, 0:O], Wp[:, 2, :, 0:O])
        nc.sync.dma_start(xp_sb[:, 7, :, 0:NT], xp[:, 7, :, 0:NT])
        nc.sync.dma_start(Wp_sb[:, 2, :, O:2 * O], Wp[:, 2, :, O:2 * O])
        nc.sync.dma_start(Wp_sb[:, 3, :, 0:O], Wp[:, 3, :, 0:O])
        nc.sync.dma_start(xp_sb[:, 2, :, NT:2 * NT], xp[:, 2, :, NT:2 * NT])
        nc.sync.dma_start(xp_sb[:, 3, :, NT:2 * NT], xp[:, 3, :, NT:2 * NT])
        nc.sync.dma_start(Wp_sb[:, 3, :, O:2 * O], Wp[:, 3, :, O:2 * O])
        for kp in range(4, KP):
            nc.sync.dma_start(xp_sb[:, kp, :, NT:2 * NT],
                              xp[:, kp, :, NT:2 * NT])
        nc.sync.dma_start(Bc_sb[:], Bc[:])
        nc.sync.dma_start(Sel_sb[:], Sel[:])
        for kp in range(4, KP):
            nc.sync.dma_start(Wp_sb[:, kp], Wp[:, kp])

        # ---- u / lg accumulators ----
        u_ps = [ps_u.tile([ER, 512], F32, name=f"ups{tb}", tag=f"u{tb}")
                for tb in range(2)]
        lg_ps = None  # created at S2 (the lg bank joins the S0/S1 rotation)

        def xs(which, kp, sl):
            off = 0 if which == "h" else NT
            return xp_sb[:, kp, :, off + sl.start:off + sl.stop]

        def emit_ulg(kp):
            st, sp = (kp == 0), (kp == KP - 1)
            for tb in range(2):
                nc.tensor.matmul(u_ps[tb][:], Ah_sb[:, kp],
                                 xs("h", kp, slice(tb * 512, (tb + 1) * 512)),
                                 start=st, stop=sp, perf_mode=DR)

        lg_holder = []

        def emit_lg_burst(ti, kp_lo=0, kp_hi=KP):
            # one sequential accumulation group per [128t, 8e] region (the
            # interp allows a single pending group per psum tile): all 8 kp
            # x 3 hi/lo terms back-to-back, then the chain consumes it.
            reg = lg_holder[0][:, ti * E:(ti + 1) * E]
            tsl = slice(ti * 128, (ti + 1) * 128)
            for kp in range(kp_lo, kp_hi):
                for t_i, (which, roff) in enumerate(
                        (("h", 0), ("h", 16), ("l", 0))):
                    nc.tensor.matmul(reg, xs(which, kp, tsl),
                                     Rr_sb[:, kp, :, roff:roff + E],
                                     start=(kp == 0 and t_i == 0),
                                     stop=(kp == KP - 1 and t_i == 2),
                                     perf_mode=DR)
            if kp_hi == KP:
                _emit_chain(nc, smallp, reg, w_tiles)

        rot = [0]
        TAGS5 = [(ps_mm, "mm0"), (ps_mm, "mm1"), (ps_mm, "mm2"),
                 (ps_mm, "mm3"), (ps_tr, "trwb"), (ps_lg, "lg")]
        TAGS6 = [(ps_mm, "mm0"), (ps_mm, "mm1"), (ps_mm, "mm2"),
                 (ps_mm, "mm3"), (ps_u, "u0"), (ps_u, "u1"),
                 (ps_tr, "trwb")]
        TAGS7 = TAGS6 + [(ps_lg, "lg")]
        TAGS8 = TAGS7 + [(ps_tr, "trwb")]

        def rot_tile(name, tags):
            pool, tag = tags[rot[0] % len(tags)]
            rot[0] += 1
            return pool.tile([128, 512], F32, name=name, tag=tag, bufs=1)

        def emit_pair_group(ti, p, kps, tags, sweep, last_sweep=False):
            """One (token-tile, ob-pair) group: per kp, 4 xh-stationary
            matmuls (Wh/Wl x 2 ob) then 2 xl-stationary (Wh x 2 ob); the
            ldweights dedupe collapses each run to one weight load.  Returns
            the 2 psum tiles (ob 2p, 2p+1)."""
            tsl = slice(ti * 128, (ti + 1) * 128)
            pss = [rot_tile(f"{sweep}_{ti}_{2 * p + i}", tags)
                   for i in range(2)]
            first_kp, last_kp = kps[0], kps[-1]
            for kp in kps:
                for i in range(2):
                    ob = 2 * p + i
                    osl = slice(ob * 512, (ob + 1) * 512)
                    nc.tensor.matmul(pss[i][:], xs("h", kp, tsl),
                                     Wp_sb[:, kp, :, osl.start:osl.stop],
                                     start=(kp == first_kp), stop=False,
                                     perf_mode=DR)
                    nc.tensor.matmul(pss[i][:], xs("h", kp, tsl),
                                     Wp_sb[:, kp, :, O + osl.start:O + osl.stop],
                                     start=False, stop=False, perf_mode=DR)
                for i in range(2):
                    ob = 2 * p + i
                    osl = slice(ob * 512, (ob + 1) * 512)
                    # the xl correction is dropped for kp 6-7: the remaining
                    # x-quantization noise on 2/8 of the contraction is
                    # ~1.3% of the output against the 2e-2 gate, and it saves
                    # 2 matmuls per (pair, kp) plus the xl bytes of the
                    # stream tail
                    if kp >= 4:
                        continue
                    nc.tensor.matmul(pss[i][:], xs("l", kp, tsl),
                                     Wp_sb[:, kp, :, osl.start:osl.stop],
                                     start=False,
                                     stop=(kp == last_kp and not last_sweep),
                                     perf_mode=DR)
            return pss

        ev = [0]
        EV_COPY = ("act", "vec", "pool", "vec")
        EV_ADD = ("vec", "pool", "vec", "vec")

        def emit_evac(ps, idx, first):
            # kp0 sweep: copy psum -> f32 partial; later sweeps: partial +=
            # psum.  ACT only ever sees copies (no tensor_tensor on the
            # scalar engine); Pool's f32 tensor ops are ~2.3x slower than
            # DVE so it takes a minority share.
            # GPSIMD/Pool cannot access PSUM on real HW (BIR verifier), so
            # copies alternate ACT/DVE and adds are DVE-only.
            dst = part_sb[:, idx * 512:(idx + 1) * 512]
            e = (EV_COPY if first else EV_ADD)[ev[0] % 4]
            ev[0] += 1
            if first:
                if e in ("act", "pool"):
                    nc.scalar.copy(dst, ps[:])
                else:
                    nc.vector.tensor_copy(dst, ps[:])
            elif e == "pool":
                # relieve DVE: ACT evacuates PSUM to a scratch tile, Pool
                # (SBUF-only) folds it into the partial
                sc = stagep.tile([128, 512], F32, name="sc", tag="sc", bufs=2)
                nc.scalar.copy(sc[:], ps[:])
                nc.gpsimd.tensor_tensor(dst, dst, sc[:], ALU.add)
            else:
                nc.vector.tensor_tensor(dst, dst, ps[:], ALU.add)

        w_tiles = []

        def emit_wexpand(tb):
            for ti in range(tb * 4, tb * 4 + 4):
                sl = slice(ti * 128, (ti + 1) * 128)
                trW = ps_tr.tile([E, 128], BF16, name="trW", tag="trwb",
                                 padded_shape=[128, 1024])
                nc.tensor.transpose(trW[:], w_tiles[ti][:], Idb_sb[:])
                nc.scalar.copy(wT_sb[:, sl], trW[:])
            sl = slice(tb * 512, (tb + 1) * 512)
            wb_ps = ps_tr.tile([ER, 512], F32, name="wbps", tag="trwb")
            nc.tensor.matmul(wb_ps[:], Mm_sb[:], wT_sb[:, sl],
                             start=True, stop=True)
            nc.vector.tensor_tensor(us_sb[:, sl], u_sb[:, sl],
                                    wb_ps[:], ALU.mult)
            # pack us rows into the DoubleRow [64, 2, t] layout for the fp8
            # finisher: PE selector matmuls move er 64..127 onto partitions
            # 0..63 (DVE cannot shift partitions), ACT casts psum -> fp8
            for j in range(2):
                pk = ps_tr.tile([64, 512], F32, name="pk", tag="trwb",
                                padded_shape=[128, 512])
                nc.tensor.matmul(pk[:], Sel_sb[:, j * 64:(j + 1) * 64],
                                 us_sb[:, sl], start=True, stop=True)
                if j == 0:
                    nc.scalar.copy(us8_sb[:, j, sl.start:sl.stop], pk[:])
                else:
                    nc.vector.tensor_copy(us8_sb[:, j, sl.start:sl.stop],
                                          pk[:])

        def emit_finish(pss4, ti, copy_stage=False):
            # fused bf16 LoRA-B finishers for all 4 ob tiles of this token
            # tile (one shared us stationary), then staging (+ partial when
            # the tile ran the early sweeps) and the store.
            tsl = slice(ti * 128, (ti + 1) * 128)
            for ob in range(OBS):
                nc.tensor.matmul(pss4[ob][:], us8_sb[:, :, tsl.start:tsl.stop],
                                 Bc_sb[:, :, ob * 512:(ob + 1) * 512],
                                 start=False, stop=True, perf_mode=DR)
            for ob in range(OBS):
                idx = ti * OBS + ob
                st = stagep.tile([128, 512], BF16, name="st", tag="st", bufs=6)
                if copy_stage:
                    if ob % 2 == 0:
                        nc.scalar.copy(st[:], pss4[ob][:])
                    else:
                        nc.vector.tensor_copy(st[:], pss4[ob][:])
                else:
                    nc.vector.tensor_tensor(st[:], pss4[ob][:],
                                            part_sb[:, idx * 512:(idx + 1) * 512],
                                            ALU.add)
                nc.sync.dma_start(out[tsl, ob * 512:(ob + 1) * 512], st[:])

        NTI = TI - 1  # ti7 skips the early sweeps: full sweep at the tail

        # ---- S0: kp0 sweep (ti 0..6); u(0) up front, u(1..2) as x lands.
        # p-major order: all ob-pair-0 groups need only the first halves of
        # the W0 stream, so PE stops chasing the W0 DMA after ~2 chunks. ----
        emit_ulg(0)

        def s0_hi(ti, pss):
            # all rows reading the Wh half of W0 (xh.Wh and xl.Wh) so the
            # prologue isn't gated on the later Wl sub-chunk
            tsl = slice(ti * 128, (ti + 1) * 128)
            for which in ("h", "l"):
                for i in range(2):
                    osl = slice(i * 512, (i + 1) * 512)
                    nc.tensor.matmul(pss[i][:], xs(which, 0, tsl),
                                     Wp_sb[:, 0, :, osl.start:osl.stop],
                                     start=(which == "h"), stop=False,
                                     perf_mode=DR)

        def s0_lo(ti, pss):
            tsl = slice(ti * 128, (ti + 1) * 128)
            for i in range(2):
                osl = slice(i * 512, (i + 1) * 512)
                nc.tensor.matmul(pss[i][:], xs("h", 0, tsl),
                                 Wp_sb[:, 0, :, O + osl.start:O + osl.stop],
                                 start=False, stop=True, perf_mode=DR)

        g0 = [rot_tile("s0e_0_%d" % i, TAGS5) for i in range(2)]
        g1 = [rot_tile("s0e_1_%d" % i, TAGS5) for i in range(2)]
        s0_hi(0, g0)
        s0_hi(1, g1)
        s0_lo(0, g0)
        for i in range(2):
            emit_evac(g0[i], 0 * OBS + i, first=True)
        g2 = [rot_tile("s0e_2_%d" % i, TAGS5) for i in range(2)]
        s0_hi(2, g2)
        s0_lo(1, g1)
        for i in range(2):
            emit_evac(g1[i], 1 * OBS + i, first=True)
        s0_lo(2, g2)
        for i in range(2):
            emit_evac(g2[i], 2 * OBS + i, first=True)
        for p in range(2):
            for ti in range(3 if p == 0 else 0, NTI):
                if (ti, p) == (0, 1):
                    emit_ulg(1)
                if (ti, p) == (4, 1):
                    emit_ulg(2)
                pss = emit_pair_group(ti, p, [0], TAGS5, "s0")
                for i in range(2):
                    emit_evac(pss[i], ti * OBS + 2 * p + i, first=True)

        # ---- S1: kp1 sweep; u(3..7) braided by x arrival; u evac ----
        emit_ulg(3)
        for ti in range(NTI):
            for p in range(2):
                if (ti, p) == (1, 1):
                    emit_ulg(4)
                if (ti, p) == (3, 1):
                    emit_ulg(5)
                if (ti, p) == (5, 1):
                    emit_ulg(6)
                pss = emit_pair_group(ti, p, [1], TAGS5, "s1")
                for i in range(2):
                    emit_evac(pss[i], ti * OBS + 2 * p + i, first=False)
        emit_ulg(7)
        for tb in range(2):
            nc.scalar.mul(u_sb[:, tb * 512:(tb + 1) * 512], u_ps[tb][:],
                          1.0 / S_A)

        # ---- S2: kp2+3 paired sweep; lg bursts + chains; w expansions ----
        lg_holder.append(ps_lg.tile([128, 8 * E], F32, name="lg", tag="lg",
                                    padded_shape=[128, 512]))
        halves = [(b, h) for b in range(TI) for h in range(2)]
        hslot = [0]

        def emit_half():
            if hslot[0] < len(halves):
                b, h = halves[hslot[0]]
                hslot[0] += 1
                emit_lg_burst(b, h * (KP // 2), (h + 1) * (KP // 2))

        for ti in range(NTI):
            for p in range(2):
                # half-bursts (8 Ldweights ~0.84us of PE SEQ each) braided
                # one or two per group slot: each hides under the preceding
                # group's 1.28us of engine time instead of stalling it
                if (ti, p) >= (0, 1):
                    emit_half()
                if ti >= 4:
                    emit_half()
                if (ti, p) == (5, 1):
                    emit_wexpand(0)
                pss = emit_pair_group(ti, p, [2, 3], TAGS6, "s2")
                for i in range(2):
                    emit_evac(pss[i], ti * OBS + 2 * p + i, first=False)

        while hslot[0] < len(halves):
            emit_half()
        emit_wexpand(1)

        # ---- S3: ti7's first two whole-K ob groups lead (their kp0-3 rows
        # are DMA-independent, absorbing the W tail window), then the per-ti
        # kp4..7 sweeps + finishers, then ti7's deferred finishers and its
        # last two obs.  The last stores issue from the ACT/DVE queues so
        # their HWDGE work runs off the SP path in the kernel tail. ----
        tsl7 = slice(7 * 128, 8 * 128)

        def ti7_base(ob, ps):
            osl = slice(ob * 512, (ob + 1) * 512)
            for kp in range(KP):
                nc.tensor.matmul(ps[:], xs("h", kp, tsl7),
                                 Wp_sb[:, kp, :, osl.start:osl.stop],
                                 start=(kp == 0), stop=False, perf_mode=DR)
                nc.tensor.matmul(ps[:], xs("h", kp, tsl7),
                                 Wp_sb[:, kp, :, O + osl.start:O + osl.stop],
                                 start=False, stop=False, perf_mode=DR)
                if kp < 4:
                    nc.tensor.matmul(ps[:], xs("l", kp, tsl7),
                                     Wp_sb[:, kp, :, osl.start:osl.stop],
                                     start=False, stop=False, perf_mode=DR)

        def ti7_fin(ob, ps, eng):
            osl = slice(ob * 512, (ob + 1) * 512)
            nc.tensor.matmul(ps[:], us8_sb[:, :, tsl7.start:tsl7.stop],
                             Bc_sb[:, :, osl.start:osl.stop],
                             start=False, stop=True, perf_mode=DR)
            st = stagep.tile([128, 512], BF16, name="st", tag="st", bufs=6)
            if eng == "act":
                nc.scalar.copy(st[:], ps[:])
            else:
                nc.vector.tensor_copy(st[:], ps[:])
            nc.sync.dma_start(out[tsl7, osl], st[:])

        TAGS6C = [(ps_mm, "mm0"), (ps_mm, "mm1"), (ps_mm, "mm2"),
                  (ps_mm, "mm3"), (ps_lg, "lg"), (ps_tr, "trwb")]
        # ti7 ob0/ob1 park on the freed u banks (outside the rotation) so
        # their finishers can wait for us8 without blocking the loop
        t7a = ps_u.tile([128, 512], F32, name="s3f_0", tag="u0", bufs=1)
        ti7_base(0, t7a)
        t7b = ps_u.tile([128, 512], F32, name="s3f_1", tag="u1", bufs=1)
        ti7_base(1, t7b)
        for ti in range(NTI):
            pssA = emit_pair_group(ti, 0, [4, 5, 6, 7], TAGS6C, "s3",
                                   last_sweep=True)
            pssB = emit_pair_group(ti, 1, [4, 5, 6, 7], TAGS6C, "s3",
                                   last_sweep=True)
            emit_finish(pssA + pssB, ti)
        ti7_fin(0, t7a, "act")
        ti7_fin(1, t7b, "vec")
        t7c = rot_tile("s3f_2", TAGS6C)
        ti7_base(2, t7c)
        ti7_fin(2, t7c, "act")
        t7d = rot_tile("s3f_3", TAGS6C)
        ti7_base(3, t7d)
        ti7_fin(3, t7d, "vec")

        ps_mm.release()
        ps_tr.release()
        ps_lg.release()
        ps_u.release()


def _ldweights_key(inst):
    ap = inst.ins[0]
    return (str(ap), str(inst.perf_mode), str(inst.is_transpose),
            str(inst.tile_position), str(inst.tile_size))


def _dedupe_ldweights(nc):
    """Drop an InstLdweights when the PE array already holds the same
    stationary (identical weights AP, only paired matmults in between).
    The ~105ns-per-instruction PE sequencer cost of redundant weight loads
    otherwise dominates the kernel."""
    removed = 0
    for bb in nc.m.functions[0].blocks:
        keep = []
        last_key = None
        for inst in bb.instructions:
            t = type(inst).__name__
            if t == "InstLdweights":
                k = _ldweights_key(inst)
                si = inst.sync_info
                has_sync = si is not None and (list(si.on_wait) or
                                               list(si.on_update))
                if k == last_key and not has_sync:
                    removed += 1
                    continue
                last_key = k
            elif t != "InstMatmult":
                if getattr(inst, "engine", None) == mybir.EngineType.PE:
                    last_key = None
            keep.append(inst)
        bb.instructions = keep
    return removed


def build_nc():
    nc = bacc.Bacc("TRN2", target_bir_lowering=False, debug=False, num_devices=N_CORES)
    xp = nc.dram_tensor("xp", [128, KP, 2, 2 * NT], F8, kind="ExternalInput").ap()
    Wp = nc.dram_tensor("Wp", [128, KP, 2, 2 * O], F8, kind="ExternalInput").ap()
    Ah = nc.dram_tensor("Ah", [128, KP, 2, ER], F8, kind="ExternalInput").ap()
    Rr = nc.dram_tensor("Rr", [128, KP, 2, 32], F8, kind="ExternalInput").ap()
    Bc = nc.dram_tensor("Bc", [64, 2, O], F8, kind="ExternalInput").ap()
    Sel = nc.dram_tensor("Sel", [ER, ER], BF16, kind="ExternalInput").ap()
    Mm = nc.dram_tensor("Mm", [E, ER], BF16, kind="ExternalInput").ap()
    Idb = nc.dram_tensor("Idb", [128, 128], BF16, kind="ExternalInput").ap()
    out = nc.dram_tensor("out", [NT, O], BF16, kind="ExternalOutput").ap()
    with tile.TileContext(nc) as tc:
        _body(tc, nc, xp, Wp, Ah, Rr, Bc, Sel, Mm, Idb, out)
    _dedupe_ldweights(nc)
    nc.compile()
    return nc


def get_nc():
    if "nc" not in _NC_CACHE:
        _NC_CACHE["nc"] = build_nc()
    return _NC_CACHE["nc"]


F8NP = ml_dtypes.float8_e4m3


def _pack_k(aT):
    """[D, C] -> [128, KP, 2, C]: element [p, kp, j, :] holds row k=kp*256+j*128+p."""
    C = aT.shape[1]
    return np.ascontiguousarray(
        aT.reshape(KP, 2, 128, C).transpose(2, 0, 1, 3))


def _hi_lo(aT):
    hi = aT.astype(F8NP)
    lo = (aT - hi.astype(np.float32)).astype(F8NP)
    return hi, lo


def make_in_maps(x, weight, lora_A, lora_B, router_w):
    x = np.ascontiguousarray(np.asarray(x, dtype=np.float32)).reshape(N_TOK, D)
    weight = np.asarray(weight, dtype=np.float32)
    lora_A = np.asarray(lora_A, dtype=np.float32)
    lora_B = np.asarray(lora_B, dtype=np.float32)
    router_w = np.asarray(router_w, dtype=np.float32)

    WTh, WTl = _hi_lo(np.ascontiguousarray(weight.T) * S_W)
    Wpm = np.concatenate([_pack_k(WTh), _pack_k(WTl)], axis=3)
    ATh = (np.ascontiguousarray(lora_A.reshape(ER, D).T) * S_A).astype(F8NP)
    Ahm = _pack_k(ATh)
    RT = np.zeros((D, 16), dtype=np.float32)
    RT[:, :E] = router_w.T * S_R
    RTh, RTl = _hi_lo(RT)
    Rrm = np.concatenate([_pack_k(RTh), _pack_k(RTl)], axis=3)
    BcT = lora_B.transpose(0, 2, 1).reshape(ER, O) * S_W
    # DR-packed: Bc8[p, j, o] = BcT[j*64 + p, o]
    Bcm = np.ascontiguousarray(BcT.reshape(2, 64, O).transpose(1, 0, 2)).astype(F8NP)
    Selm = np.zeros((ER, ER), dtype=np.float32)
    for j in range(2):
        for m in range(64):
            Selm[j * 64 + m, j * 64 + m] = 1.0
    # lhsT selector: out[m,t] = sum_er Sel[er, j*64+m-block] us[er, t]
    Selm = Selm.astype(ml_dtypes.bfloat16)
    Mmm = np.zeros((E, ER), dtype=np.float32)
    for e in range(E):
        Mmm[e, e * R:(e + 1) * R] = 1.0
    Mmm = Mmm.astype(ml_dtypes.bfloat16)
    Idb = np.eye(128, dtype=np.float32).astype(ml_dtypes.bfloat16)

    in_maps = []
    for c in range(N_CORES):
        xT = np.ascontiguousarray(x[c * NT:(c + 1) * NT].T)
        xTh, xTl = _hi_lo(xT)
        in_maps.append({
            "xp": np.concatenate([_pack_k(xTh), _pack_k(xTl)], axis=3),
            "Wp": Wpm,
            "Ah": Ahm,
            "Rr": Rrm,
            "Bc": Bcm,
            "Sel": Selm,
            "Mm": Mmm,
            "Idb": Idb,
        })
    return in_maps


def kernel(x, weight, lora_A, lora_B, router_w):
    global LAST_RESULTS
    from concourse.bass_utils import run_bass_kernel_spmd

    in_maps = make_in_maps(x, weight, lora_A, lora_B, router_w)
    nc = get_nc()
    trace = bool(os.environ.get("KBENCH_TRACE"))
    res = run_bass_kernel_spmd(nc, in_maps, core_ids=list(range(N_CORES)), trace=trace)
    LAST_RESULTS = res
    outs = [np.asarray(res.results[c]["out"], dtype=np.float32) * (1.0 / S_W)
            for c in range(N_CORES)]
    return np.concatenate(outs, axis=0).reshape(4, 2048, 2048)


# revision 69
# speedup vs baseline: 1.0126x; 1.0084x over previous
"""LoRA-MoE layer (base dense + top-2 routed rank-16 LoRA experts) on 8 TRN2 cores.

Data-parallel over tokens (8192 -> 1024/core), weights replicated, zero
collectives.  The dense base projection runs as fp8e4 DoubleRow matmuls
(K=256 packed per instruction, 2 output columns/cycle) with a hi/lo split:
    x@W ~= xh@Wh + xh@Wl + xl@Wh      xh = fp8(x), xl = fp8(x - xh)
W/A/R are pre-scaled on host (x512/x32/x32) so their uniform(+-1/sqrt(2048))
values escape fp8's subnormal range; the W-scale rides through PSUM (Bc
carries it too) and is divided out on the host after gather.  The xl
correction is dropped for kp>=4 (half the contraction): measured full-batch
error 1.755e-2 against the 2e-2 gate, for 25% fewer base matmuls.

PE-sequencer economics dominate instruction choice: every Ldweights costs
~105ns of PE SEQ while a matmult costs ~2ns, so matmuls are grouped
(ti, ob-pair)-major so runs of 4-6 share one stationary, and a post-pass
(_dedupe_ldweights) deletes the redundant weight loads the tile scheduler
emits 1:1.

Schedule: 8 k-pair chunks; PSUM holds only ~6 rotating accumulators, so
sweeps S0 (kp0), S1 (kp1), S2 (kp2+3) evacuate per-tile partials into an
SBUF f32 accumulator (ACT/DVE copies + DVE adds - GPSIMD cannot touch PSUM
on real HW; a minority of adds go ACT-copy + Pool SBUF-add), and S3 (kp4-7)
fuses the fp8-DR LoRA-B finisher and a staging add before each store.  Token
tile 7 skips the early sweeps entirely and runs whole-K per-ob groups at the
tail, storing each ob as soon as it finishes.  The DMA stream is ordered so
every sweep's W chunks land just ahead of their consumers (W0 halves split
finer for the prologue, W1 behind x1, W2/W3 braided into the x tail, W4-7
last); merged hi/lo tensors keep the 625ns-per-descriptor HWDGE issue rate
off the critical path.

Routing: per (ti, kp) three ap-8 DoubleRow matmuls (xh.Rh + xh.Rl + xl.Rh)
ride the base x stationaries, accumulating token-major [128t, 8e] regions of
one PSUM bank as sequential per-ti bursts; the top-2 softmax chains (DVE/ACT)
consume them directly - no logit transposes.  u = A.x is 1-term fp8 DR.
Per-token weights are transposed and expanded to [er, t] via the one-hot Mm
matmul; us = u * wb is then re-packed to the DoubleRow [64, 2, t] layout with
PE selector matmuls (DVE cannot shift partitions) so the finisher runs fp8.
"""

import os
import sys

import numpy as np


def _ensure_concourse():
    try:
        import concourse  # noqa: F401
    except ImportError:
        for p in ("/opt/trn_rl_repo", os.path.expanduser("~/.axon_site/_ro/trn_rl_repo")):
            if os.path.isdir(p):
                sys.path.insert(0, p)
                break


_ensure_concourse()

import ml_dtypes  # noqa: E402
import concourse.bass as bass  # noqa: E402,F401
import concourse.tile as tile  # noqa: E402
from concourse import bacc, mybir  # noqa: E402

F32 = mybir.dt.float32
BF16 = mybir.dt.bfloat16
F8 = mybir.dt.float8e4
X_AX = mybir.AxisListType.X
ALU = mybir.AluOpType
ACT = mybir.ActivationFunctionType
DR = mybir.MatmulPerfMode.DoubleRow

N_CORES = 8
N_TOK = 8192          # total tokens (4 x 2048)
NT = N_TOK // N_CORES  # tokens per core = 1024
D = 2048
O = 2048
E = 8
R = 16
ER = E * R            # 128
KP = D // 256         # 8 k-pair chunks (256 contraction each, DoubleRow-packed)
TI = NT // 128        # 8 token tiles
OBS = 4               # o blocks of 512

S_W = 512.0           # host scale on W (and Bc); divided out on host
S_A = 32.0            # host scale on lora_A; divided out at u evac
S_R = 32.0            # host scale on router_w; divided out at chain head

_NC_CACHE = {}
LAST_RESULTS = None


def _emit_chain(nc, smallp, lg_reg, w_tiles):
    """Top-2 softmax weight chain for one 128-token tile (DVE/ACT ops).

    lg_reg is the token-major [128, 8] f32 PSUM region holding S_R * logits."""
    L = smallp.tile([128, E], F32, name="L", tag="L")
    nc.scalar.mul(L[:], lg_reg, 1.0 / S_R)
    m1 = smallp.tile([128, 1], F32, name="m1", tag="m1")
    nc.vector.reduce_max(m1[:], L[:], axis=X_AX)
    nm1 = smallp.tile([128, 1], F32, name="nm1", tag="nm1")
    nc.scalar.mul(nm1[:], m1[:], -1.0)
    # mask out the top-1 entry, then find the 2nd max
    msk = smallp.tile([128, E], F32, name="msk", tag="msk")
    nc.vector.tensor_scalar(msk[:], L[:], m1[:], -1e30, ALU.is_equal, ALU.mult)
    L2 = smallp.tile([128, E], F32, name="L2", tag="L2")
    nc.vector.tensor_tensor(L2[:], L[:], msk[:], ALU.add)
    m2 = smallp.tile([128, 1], F32, name="m2", tag="m2")
    nc.vector.reduce_max(m2[:], L2[:], axis=X_AX)
    eL = smallp.tile([128, E], F32, name="eL", tag="eL")
    nc.scalar.activation(eL[:], L[:], ACT.Exp, bias=nm1[:])
    ge = smallp.tile([128, E], F32, name="ge", tag="ge")
    nc.vector.tensor_scalar(ge[:], L[:], m2[:], None, ALU.is_ge)
    un = smallp.tile([128, E], F32, name="un", tag="un")
    nc.vector.tensor_tensor(un[:], eL[:], ge[:], ALU.mult)
    s = smallp.tile([128, 1], F32, name="s", tag="s")
    nc.vector.reduce_sum(s[:], un[:], axis=X_AX)
    r = smallp.tile([128, 1], F32, name="r", tag="r")
    nc.vector.reciprocal(r[:], s[:])
    r2 = smallp.tile([128, 1], F32, name="r2", tag="r2")
    nc.scalar.mul(r2[:], r[:], 2.0)  # fold SCALING = 2.0
    w = smallp.tile([128, E], BF16, name="w", tag="w", bufs=8)
    nc.vector.tensor_scalar(w[:], un[:], r2[:], None, ALU.mult)
    w_tiles.append(w)


def _body(tc, nc, xp, Wp, Ah, Rr, Bc, Sel, Mm, Idb, out):
    with (
        tc.tile_pool(name="const", bufs=1) as constp,
        tc.tile_pool(name="small", bufs=4) as smallp,
        tc.tile_pool(name="stage", bufs=4) as stagep,
    ):
        # PSUM: u(2) + lg(1) + rotation mm0..3 (4) + trwb (1) = 8 banks.
        ps_u = tc.alloc_tile_pool(name="ps_u", bufs=1, space="PSUM")
        ps_lg = tc.alloc_tile_pool(name="ps_lg", bufs=1, space="PSUM")
        ps_tr = tc.alloc_tile_pool(name="ps_tr", bufs=1, space="PSUM")
        ps_mm = tc.alloc_tile_pool(name="ps_mm", bufs=1, space="PSUM")

        # ---- resident SBUF tensors ----
        xp_sb = constp.tile([128, KP, 2, 2 * NT], F8, name="xp_sb")
        Wp_sb = constp.tile([128, KP, 2, 2 * O], F8, name="Wp_sb")
        Ah_sb = constp.tile([128, KP, 2, ER], F8, name="Ah_sb")
        Rr_sb = constp.tile([128, KP, 2, 32], F8, name="Rr_sb")
        Bc_sb = constp.tile([64, 2, O], F8, name="Bc_sb")
        Sel_sb = constp.tile([ER, ER], BF16, name="Sel_sb")
        us8_sb = constp.tile([64, 2, NT], F8, name="us8_sb")
        Mm_sb = constp.tile([E, ER], BF16, name="Mm_sb")
        Idb_sb = constp.tile([128, 128], BF16, name="Idb_sb")
        u_sb = constp.tile([ER, NT], F32, name="u_sb")
        us_sb = constp.tile([ER, NT], BF16, name="us_sb")
        wT_sb = constp.tile([E, NT], BF16, name="wT_sb")
        part_sb = constp.tile([128, 32 * 512], F32, name="part_sb")

        tiles = [(ti, ob) for ti in range(TI) for ob in range(OBS)]  # 32

        # PE p-state warm-up: the cost model runs PE at 0.65-1.2GHz for the
        # first 3us of busy time.  Burn the ramp on throwaway fp32 matmuls
        # (iota-seeded, no DMA dependency) while the first chunks stream in.
        # PE p-state warm-up: the model halves PE speed for the first 3us
        # of busy time; burn most of the ramp on throwaway matmuls while the
        # first DMA chunks land (out partitions = wu free dim = 8).
        wu_sb = constp.tile([128, 520], F32, name="wu_sb")
        wu_ps = ps_lg.tile([8, 512], F32, name="wu_ps", tag="lg",
                           padded_shape=[128, 512])
        nc.gpsimd.memset(wu_sb[:], 1.0)
        nc.tensor.matmul(wu_ps[:], wu_sb[:, 0:8], wu_sb[:, 8:520],
                         start=True, stop=True)

        # ---- DMA emission (single in-order SP/HWDGE stream) ----
        # One dma_start per merged hi/lo chunk (the 625ns HWDGE issue cost
        # otherwise rate-limits).  First x/W chunks split finer so PE starts
        # ~3us in; W kp1 behind x kp1 (S0/S1 never W-gated); W2 early, then
        # the x tail, W3 just ahead of S2's kp3 rows, Bc, and the W tail
        # ahead of S3.
        # first chunks issue from the ACT queue in parallel with SP's so
        # both DGE pipelines fill while the DMA engines are still empty
        nc.sync.dma_start(Ah_sb[:], Ah[:])
        nc.sync.dma_start(xp_sb[:, 0, :, 0:512], xp[:, 0, :, 0:512])
        nc.sync.dma_start(xp_sb[:, 0, :, 512:1024], xp[:, 0, :, 512:1024])
        nc.scalar.dma_start(Wp_sb[:, 0, :, 0:1024], Wp[:, 0, :, 0:1024])
        nc.sync.dma_start(xp_sb[:, 0, :, 1024:2048], xp[:, 0, :, 1024:2048])
        nc.sync.dma_start(Wp_sb[:, 0, :, 2048:3072], Wp[:, 0, :, 2048:3072])
        nc.sync.dma_start(Rr_sb[:], Rr[:])
        nc.sync.dma_start(Wp_sb[:, 0, :, 1024:2048], Wp[:, 0, :, 1024:2048])
        nc.sync.dma_start(Wp_sb[:, 0, :, 3072:4096], Wp[:, 0, :, 3072:4096])
        nc.sync.dma_start(Mm_sb[:], Mm[:])
        nc.sync.dma_start(Idb_sb[:], Idb[:])
        nc.sync.dma_start(xp_sb[:, 1], xp[:, 1])
        nc.sync.dma_start(Wp_sb[:, 1], Wp[:, 1])
        # x hi-halves first (base sweeps + u read only xh); the lo (xl)
        # halves of kp>=4 feed nothing but the S2 logit bursts, so they ship
        # after W3 - pulling every W chunk ~3us earlier
        for kp in range(2, 6):
            nc.sync.dma_start(xp_sb[:, kp, :, 0:NT], xp[:, kp, :, 0:NT])
        nc.sync.dma_start(xp_sb[:, 6, :, 0:NT], xp[:, 6, :, 0:NT])
        nc.sync.dma_start(Wp_sb[:, 2, :, 0:O], Wp[:, 2, :, 0:O])
        nc.sync.dma_start(xp_sb[:, 7, :, 0:NT], xp[:, 7, :, 0:NT])
        nc.sync.dma_start(Wp_sb[:, 2, :, O:2 * O], Wp[:, 2, :, O:2 * O])
        nc.sync.dma_start(Wp_sb[:, 3, :, 0:O], Wp[:, 3, :, 0:O])
        nc.sync.dma_start(xp_sb[:, 2, :, NT:2 * NT], xp[:, 2, :, NT:2 * NT])
        nc.sync.dma_start(xp_sb[:, 3, :, NT:2 * NT], xp[:, 3, :, NT:2 * NT])
        nc.sync.dma_start(Wp_sb[:, 3, :, O:2 * O], Wp[:, 3, :, O:2 * O])
        for kp in range(4, KP):
            nc.sync.dma_start(xp_sb[:, kp, :, NT:2 * NT],
                              xp[:, kp, :, NT:2 * NT])
        nc.sync.dma_start(Bc_sb[:], Bc[:])
        nc.sync.dma_start(Sel_sb[:], Sel[:])
        for kp in range(4, KP):
            nc.sync.dma_start(Wp_sb[:, kp], Wp[:, kp])

        # ---- u / lg accumulators ----
        u_ps = [ps_u.tile([ER, 512], F32, name=f"ups{tb}", tag=f"u{tb}")
                for tb in range(2)]
        lg_ps = None  # created at S2 (the lg bank joins the S0/S1 rotation)

        def xs(which, kp, sl):
            off = 0 if which == "h" else NT
            return xp_sb[:, kp, :, off + sl.start:off + sl.stop]

        def emit_ulg(kp):
            st, sp = (kp == 0), (kp == KP - 1)
            for tb in range(2):
                nc.tensor.matmul(u_ps[tb][:], Ah_sb[:, kp],
                                 xs("h", kp, slice(tb * 512, (tb + 1) * 512)),
                                 start=st, stop=sp, perf_mode=DR)

        lg_holder = []

        def emit_lg_burst(ti, kp_lo=0, kp_hi=KP):
            # one sequential accumulation group per [128t, 8e] region (the
            # interp allows a single pending group per psum tile): all 8 kp
            # x 3 hi/lo terms back-to-back, then the chain consumes it.
            reg = lg_holder[0][:, ti * E:(ti + 1) * E]
            tsl = slice(ti * 128, (ti + 1) * 128)
            for kp in range(kp_lo, kp_hi):
                for t_i, (which, roff) in enumerate(
                        (("h", 0), ("h", 16), ("l", 0))):
                    nc.tensor.matmul(reg, xs(which, kp, tsl),
                                     Rr_sb[:, kp, :, roff:roff + E],
                                     start=(kp == 0 and t_i == 0),
                                     stop=(kp == KP - 1 and t_i == 2),
                                     perf_mode=DR)
            if kp_hi == KP:
                _emit_chain(nc, smallp, reg, w_tiles)

        rot = [0]
        TAGS5 = [(ps_mm, "mm0"), (ps_mm, "mm1"), (ps_mm, "mm2"),
                 (ps_mm, "mm3"), (ps_tr, "trwb"), (ps_lg, "lg")]
        TAGS6 = [(ps_mm, "mm0"), (ps_mm, "mm1"), (ps_mm, "mm2"),
                 (ps_mm, "mm3"), (ps_u, "u0"), (ps_u, "u1"),
                 (ps_tr, "trwb")]
        TAGS7 = TAGS6 + [(ps_lg, "lg")]
        TAGS8 = TAGS7 + [(ps_tr, "trwb")]

        def rot_tile(name, tags):
            pool, tag = tags[rot[0] % len(tags)]
            rot[0] += 1
            return pool.tile([128, 512], F32, name=name, tag=tag, bufs=1)

        def emit_pair_group(ti, p, kps, tags, sweep, last_sweep=False):
            """One (token-tile, ob-pair) group: per kp, 4 xh-stationary
            matmuls (Wh/Wl x 2 ob) then 2 xl-stationary (Wh x 2 ob); the
            ldweights dedupe collapses each run to one weight load.  Returns
            the 2 psum tiles (ob 2p, 2p+1)."""
            tsl = slice(ti * 128, (ti + 1) * 128)
            pss = [rot_tile(f"{sweep}_{ti}_{2 * p + i}", tags)
                   for i in range(2)]
            first_kp, last_kp = kps[0], kps[-1]
            for kp in kps:
                for i in range(2):
                    ob = 2 * p + i
                    osl = slice(ob * 512, (ob + 1) * 512)
                    nc.tensor.matmul(pss[i][:], xs("h", kp, tsl),
                                     Wp_sb[:, kp, :, osl.start:osl.stop],
                                     start=(kp == first_kp), stop=False,
                                     perf_mode=DR)
                    nc.tensor.matmul(pss[i][:], xs("h", kp, tsl),
                                     Wp_sb[:, kp, :, O + osl.start:O + osl.stop],
                                     start=False, stop=False, perf_mode=DR)
                for i in range(2):
                    ob = 2 * p + i
                    osl = slice(ob * 512, (ob + 1) * 512)
                    # the xl correction is dropped for kp 6-7: the remaining
                    # x-quantization noise on 2/8 of the contraction is
                    # ~1.3% of the output against the 2e-2 gate, and it saves
                    # 2 matmuls per (pair, kp) plus the xl bytes of the
                    # stream tail
                    if kp >= 4:
                        continue
                    nc.tensor.matmul(pss[i][:], xs("l", kp, tsl),
                                     Wp_sb[:, kp, :, osl.start:osl.stop],
                                     start=False,
                                     stop=(kp == last_kp and not last_sweep),
                                     perf_mode=DR)
            return pss

        ev = [0]
        EV_COPY = ("act", "vec", "pool", "vec")
        EV_ADD = ("vec", "pool", "vec", "vec")

        def emit_evac(ps, idx, first):
            # kp0 sweep: copy psum -> f32 partial; later sweeps: partial +=
            # psum.  ACT only ever sees copies (no tensor_tensor on the
            # scalar engine); Pool's f32 tensor ops are ~2.3x slower than
            # DVE so it takes a minority share.
            # GPSIMD/Pool cannot access PSUM on real HW (BIR verifier), so
            # copies alternate ACT/DVE and adds are DVE-only.
            dst = part_sb[:, idx * 512:(idx + 1) * 512]
            e = (EV_COPY if first else EV_ADD)[ev[0] % 4]
            ev[0] += 1
            if first:
                if e in ("act", "pool"):
                    nc.scalar.copy(dst, ps[:])
                else:
                    nc.vector.tensor_copy(dst, ps[:])
            elif e == "pool":
                # relieve DVE: ACT evacuates PSUM to a scratch tile, Pool
                # (SBUF-only) folds it into the partial
                sc = stagep.tile([128, 512], F32, name="sc", tag="sc", bufs=2)
                nc.scalar.copy(sc[:], ps[:])
                nc.gpsimd.tensor_tensor(dst, dst, sc[:], ALU.add)
            else:
                nc.vector.tensor_tensor(dst, dst, ps[:], ALU.add)

        w_tiles = []

        def emit_wexpand(tb):
            for ti in range(tb * 4, tb * 4 + 4):
                sl = slice(ti * 128, (ti + 1) * 128)
                trW = ps_tr.tile([E, 128], BF16, name="trW", tag="trwb",
                                 padded_shape=[128, 1024])
                nc.tensor.transpose(trW[:], w_tiles[ti][:], Idb_sb[:])
                nc.scalar.copy(wT_sb[:, sl], trW[:])
            sl = slice(tb * 512, (tb + 1) * 512)
            wb_ps = ps_tr.tile([ER, 512], F32, name="wbps", tag="trwb")
            nc.tensor.matmul(wb_ps[:], Mm_sb[:], wT_sb[:, sl],
                             start=True, stop=True)
            nc.vector.tensor_tensor(us_sb[:, sl], u_sb[:, sl],
                                    wb_ps[:], ALU.mult)
            # pack us rows into the DoubleRow [64, 2, t] layout for the fp8
            # finisher: PE selector matmuls move er 64..127 onto partitions
            # 0..63 (DVE cannot shift partitions), ACT casts psum -> fp8
            for j in range(2):
                pk = ps_tr.tile([64, 512], F32, name="pk", tag="trwb",
                                padded_shape=[128, 512])
                nc.tensor.matmul(pk[:], Sel_sb[:, j * 64:(j + 1) * 64],
                                 us_sb[:, sl], start=True, stop=True)
                if j == 0:
                    nc.scalar.copy(us8_sb[:, j, sl.start:sl.stop], pk[:])
                else:
                    nc.vector.tensor_copy(us8_sb[:, j, sl.start:sl.stop],
                                          pk[:])

        def emit_finish(pss4, ti, copy_stage=False):
            # fused bf16 LoRA-B finishers for all 4 ob tiles of this token
            # tile (one shared us stationary), then staging (+ partial when
            # the tile ran the early sweeps) and the store.
            tsl = slice(ti * 128, (ti + 1) * 128)
            for ob in range(OBS):
                nc.tensor.matmul(pss4[ob][:], us8_sb[:, :, tsl.start:tsl.stop],
                                 Bc_sb[:, :, ob * 512:(ob + 1) * 512],
                                 start=False, stop=True, perf_mode=DR)
            for ob in range(OBS):
                idx = ti * OBS + ob
                st = stagep.tile([128, 512], BF16, name="st", tag="st", bufs=6)
                if copy_stage:
                    if ob % 2 == 0:
                        nc.scalar.copy(st[:], pss4[ob][:])
                    else:
                        nc.vector.tensor_copy(st[:], pss4[ob][:])
                else:
                    nc.vector.tensor_tensor(st[:], pss4[ob][:],
                                            part_sb[:, idx * 512:(idx + 1) * 512],
                                            ALU.add)
                nc.sync.dma_start(out[tsl, ob * 512:(ob + 1) * 512], st[:])

        NTI = TI - 1  # ti7 skips the early sweeps: full sweep at the tail

        # ---- S0: kp0 sweep (ti 0..6); u(0) up front, u(1..2) as x lands.
        # p-major order: all ob-pair-0 groups need only the first halves of
        # the W0 stream, so PE stops chasing the W0 DMA after ~2 chunks. ----
        emit_ulg(0)

        def s0_hi(ti, pss):
            # all rows reading the Wh half of W0 (xh.Wh and xl.Wh) so the
            # prologue isn't gated on the later Wl sub-chunk
            tsl = slice(ti * 128, (ti + 1) * 128)
            for which in ("h", "l"):
                for i in range(2):
                    osl = slice(i * 512, (i + 1) * 512)
                    nc.tensor.matmul(pss[i][:], xs(which, 0, tsl),
                                     Wp_sb[:, 0, :, osl.start:osl.stop],
                                     start=(which == "h"), stop=False,
                                     perf_mode=DR)

        def s0_lo(ti, pss):
            tsl = slice(ti * 128, (ti + 1) * 128)
            for i in range(2):
                osl = slice(i * 512, (i + 1) * 512)
                nc.tensor.matmul(pss[i][:], xs("h", 0, tsl),
                                 Wp_sb[:, 0, :, O + osl.start:O + osl.stop],
                                 start=False, stop=True, perf_mode=DR)

        g0 = [rot_tile("s0e_0_%d" % i, TAGS5) for i in range(2)]
        g1 = [rot_tile("s0e_1_%d" % i, TAGS5) for i in range(2)]
        s0_hi(0, g0)
        s0_hi(1, g1)
        s0_lo(0, g0)
        for i in range(2):
            emit_evac(g0[i], 0 * OBS + i, first=True)
        g2 = [rot_tile("s0e_2_%d" % i, TAGS5) for i in range(2)]
        s0_hi(2, g2)
        s0_lo(1, g1)
        for i in range(2):
            emit_evac(g1[i], 1 * OBS + i, first=True)
        s0_lo(2, g2)
        for i in range(2):
            emit_evac(g2[i], 2 * OBS + i, first=True)
        for p in range(2):
            for ti in range(3 if p == 0 else 0, NTI):
                if (ti, p) == (0, 1):
                    emit_ulg(1)
                if (ti, p) == (4, 1):
                    emit_ulg(2)
                pss = emit_pair_group(ti, p, [0], TAGS5, "s0")
                for i in range(2):
                    emit_evac(pss[i], ti * OBS + 2 * p + i, first=True)

        # ---- S1: kp1 sweep; u(3..7) braided by x arrival; u evac ----
        emit_ulg(3)
        for ti in range(NTI):
            for p in range(2):
                if (ti, p) == (1, 1):
                    emit_ulg(4)
                if (ti, p) == (3, 1):
                    emit_ulg(5)
                if (ti, p) == (5, 1):
                    emit_ulg(6)
                pss = emit_pair_group(ti, p, [1], TAGS5, "s1")
                for i in range(2):
                    emit_evac(pss[i], ti * OBS + 2 * p + i, first=False)
        emit_ulg(7)
        for tb in range(2):
            nc.scalar.mul(u_sb[:, tb * 512:(tb + 1) * 512], u_ps[tb][:],
                          1.0 / S_A)

        # ---- S2: kp2+3 paired sweep; lg bursts + chains; w expansions ----
        lg_holder.append(ps_lg.tile([128, 8 * E], F32, name="lg", tag="lg",
                                    padded_shape=[128, 512]))
        quarters = [(b, q) for b in range(TI) for q in range(4)]
        hslot = [0]

        def emit_quarter():
            if hslot[0] < len(quarters):
                b, q = quarters[hslot[0]]
                hslot[0] += 1
                emit_lg_burst(b, 2 * q, 2 * q + 2)

        for ti in range(NTI):
            for p in range(2):
                # quarter-bursts (4 Ldweights ~0.42us of PE SEQ each)
                # braided 2-3 per group slot: each slot's sequencer load
                # stays under the group's 1.28us engine shadow
                if (ti, p) >= (0, 1):
                    emit_quarter()
                    emit_quarter()
                if ti >= 4:
                    emit_quarter()
                if (ti, p) == (4, 1):
                    emit_wexpand(0)
                pss = emit_pair_group(ti, p, [2, 3], TAGS6, "s2")
                for i in range(2):
                    emit_evac(pss[i], ti * OBS + 2 * p + i, first=False)

        while hslot[0] < len(quarters):
            emit_quarter()
        emit_wexpand(1)

        # ---- S3: ti7's first two whole-K ob groups lead (their kp0-3 rows
        # are DMA-independent, absorbing the W tail window), then the per-ti
        # kp4..7 sweeps + finishers, then ti7's deferred finishers and its
        # last two obs.  The last stores issue from the ACT/DVE queues so
        # their HWDGE work runs off the SP path in the kernel tail. ----
        tsl7 = slice(7 * 128, 8 * 128)

        def ti7_base(ob, ps):
            osl = slice(ob * 512, (ob + 1) * 512)
            for kp in range(KP):
                nc.tensor.matmul(ps[:], xs("h", kp, tsl7),
                                 Wp_sb[:, kp, :, osl.start:osl.stop],
                                 start=(kp == 0), stop=False, perf_mode=DR)
                nc.tensor.matmul(ps[:], xs("h", kp, tsl7),
                                 Wp_sb[:, kp, :, O + osl.start:O + osl.stop],
                                 start=False, stop=False, perf_mode=DR)
                if kp < 4:
                    nc.tensor.matmul(ps[:], xs("l", kp, tsl7),
                                     Wp_sb[:, kp, :, osl.start:osl.stop],
                                     start=False, stop=False, perf_mode=DR)

        def ti7_fin(ob, ps, eng):
            osl = slice(ob * 512, (ob + 1) * 512)
            nc.tensor.matmul(ps[:], us8_sb[:, :, tsl7.start:tsl7.stop],
                             Bc_sb[:, :, osl.start:osl.stop],
                             start=False, stop=True, perf_mode=DR)
            st = stagep.tile([128, 512], BF16, name="st", tag="st", bufs=6)
            if eng == "act":
                nc.scalar.copy(st[:], ps[:])
            else:
                nc.vector.tensor_copy(st[:], ps[:])
            nc.sync.dma_start(out[tsl7, osl], st[:])

        TAGS6C = [(ps_mm, "mm0"), (ps_mm, "mm1"), (ps_mm, "mm2"),
                  (ps_mm, "mm3"), (ps_lg, "lg"), (ps_tr, "trwb")]
        # ti7 ob0/ob1 park on the freed u banks (outside the rotation) so
        # their finishers can wait for us8 without blocking the loop
        t7a = ps_u.tile([128, 512], F32, name="s3f_0", tag="u0", bufs=1)
        ti7_base(0, t7a)
        t7b = ps_u.tile([128, 512], F32, name="s3f_1", tag="u1", bufs=1)
        ti7_base(1, t7b)
        for ti in range(NTI):
            pssA = emit_pair_group(ti, 0, [4, 5, 6, 7], TAGS6C, "s3",
                                   last_sweep=True)
            pssB = emit_pair_group(ti, 1, [4, 5, 6, 7], TAGS6C, "s3",
                                   last_sweep=True)
            emit_finish(pssA + pssB, ti)
        ti7_fin(0, t7a, "act")
        ti7_fin(1, t7b, "vec")
        t7c = rot_tile("s3f_2", TAGS6C)
        ti7_base(2, t7c)
        ti7_fin(2, t7c, "act")
        t7d = rot_tile("s3f_3", TAGS6C)
        ti7_base(3, t7d)
        ti7_fin(3, t7d, "vec")

        ps_mm.release()
        ps_tr.release()
        ps_lg.release()
        ps_u.release()


def _ldweights_key(inst):
    ap = inst.ins[0]
    return (str(ap), str(inst.perf_mode), str(inst.is_transpose),
            str(inst.tile_position), str(inst.tile_size))


def _dedupe_ldweights(nc):
    """Drop an InstLdweights when the PE array already holds the same
    stationary (identical weights AP, only paired matmults in between).
    The ~105ns-per-instruction PE sequencer cost of redundant weight loads
    otherwise dominates the kernel."""
    removed = 0
    for bb in nc.m.functions[0].blocks:
        keep = []
        last_key = None
        for inst in bb.instructions:
            t = type(inst).__name__
            if t == "InstLdweights":
                k = _ldweights_key(inst)
                si = inst.sync_info
                has_sync = si is not None and (list(si.on_wait) or
                                               list(si.on_update))
                if k == last_key and not has_sync:
                    removed += 1
                    continue
                last_key = k
            elif t != "InstMatmult":
                if getattr(inst, "engine", None) == mybir.EngineType.PE:
                    last_key = None
            keep.append(inst)
        bb.instructions = keep
    return removed


def build_nc():
    nc = bacc.Bacc("TRN2", target_bir_lowering=False, debug=False, num_devices=N_CORES)
    xp = nc.dram_tensor("xp", [128, KP, 2, 2 * NT], F8, kind="ExternalInput").ap()
    Wp = nc.dram_tensor("Wp", [128, KP, 2, 2 * O], F8, kind="ExternalInput").ap()
    Ah = nc.dram_tensor("Ah", [128, KP, 2, ER], F8, kind="ExternalInput").ap()
    Rr = nc.dram_tensor("Rr", [128, KP, 2, 32], F8, kind="ExternalInput").ap()
    Bc = nc.dram_tensor("Bc", [64, 2, O], F8, kind="ExternalInput").ap()
    Sel = nc.dram_tensor("Sel", [ER, ER], BF16, kind="ExternalInput").ap()
    Mm = nc.dram_tensor("Mm", [E, ER], BF16, kind="ExternalInput").ap()
    Idb = nc.dram_tensor("Idb", [128, 128], BF16, kind="ExternalInput").ap()
    out = nc.dram_tensor("out", [NT, O], BF16, kind="ExternalOutput").ap()
    with tile.TileContext(nc) as tc:
        _body(tc, nc, xp, Wp, Ah, Rr, Bc, Sel, Mm, Idb, out)
    _dedupe_ldweights(nc)
    nc.compile()
    return nc


def get_nc():
    if "nc" not in _NC_CACHE:
        _NC_CACHE["nc"] = build_nc()
    return _NC_CACHE["nc"]


F8NP = ml_dtypes.float8_e4m3


def _pack_k(aT):
    """[D, C] -> [128, KP, 2, C]: element [p, kp, j, :] holds row k=kp*256+j*128+p."""
    C = aT.shape[1]
    return np.ascontiguousarray(
        aT.reshape(KP, 2, 128, C).transpose(2, 0, 1, 3))


def _hi_lo(aT):
    hi = aT.astype(F8NP)
    lo = (aT - hi.astype(np.float32)).astype(F8NP)
    return hi, lo


def make_in_maps(x, weight, lora_A, lora_B, router_w):
    x = np.ascontiguousarray(np.asarray(x, dtype=np.float32)).reshape(N_TOK, D)
    weight = np.asarray(weight, dtype=np.float32)
    lora_A = np.asarray(lora_A, dtype=np.float32)
    lora_B = np.asarray(lora_B, dtype=np.float32)
    router_w = np.asarray(router_w, dtype=np.float32)

    WTh, WTl = _hi_lo(np.ascontiguousarray(weight.T) * S_W)
    Wpm = np.concatenate([_pack_k(WTh), _pack_k(WTl)], axis=3)
    ATh = (np.ascontiguousarray(lora_A.reshape(ER, D).T) * S_A).astype(F8NP)
    Ahm = _pack_k(ATh)
    RT = np.zeros((D, 16), dtype=np.float32)
    RT[:, :E] = router_w.T * S_R
    RTh, RTl = _hi_lo(RT)
    Rrm = np.concatenate([_pack_k(RTh), _pack_k(RTl)], axis=3)
    BcT = lora_B.transpose(0, 2, 1).reshape(ER, O) * S_W
    # DR-packed: Bc8[p, j, o] = BcT[j*64 + p, o]
    Bcm = np.ascontiguousarray(BcT.reshape(2, 64, O).transpose(1, 0, 2)).astype(F8NP)
    Selm = np.zeros((ER, ER), dtype=np.float32)
    for j in range(2):
        for m in range(64):
            Selm[j * 64 + m, j * 64 + m] = 1.0
    # lhsT selector: out[m,t] = sum_er Sel[er, j*64+m-block] us[er, t]
    Selm = Selm.astype(ml_dtypes.bfloat16)
    Mmm = np.zeros((E, ER), dtype=np.float32)
    for e in range(E):
        Mmm[e, e * R:(e + 1) * R] = 1.0
    Mmm = Mmm.astype(ml_dtypes.bfloat16)
    Idb = np.eye(128, dtype=np.float32).astype(ml_dtypes.bfloat16)

    in_maps = []
    for c in range(N_CORES):
        xT = np.ascontiguousarray(x[c * NT:(c + 1) * NT].T)
        xTh, xTl = _hi_lo(xT)
        in_maps.append({
            "xp": np.concatenate([_pack_k(xTh), _pack_k(xTl)], axis=3),
            "Wp": Wpm,
            "Ah": Ahm,
            "Rr": Rrm,
            "Bc": Bcm,
            "Sel": Selm,
            "Mm": Mmm,
            "Idb": Idb,
        })
    return in_maps


def kernel(x, weight, lora_A, lora_B, router_w):
    global LAST_RESULTS
    from concourse.bass_utils import run_bass_kernel_spmd

    in_maps = make_in_maps(x, weight, lora_A, lora_B, router_w)
    nc = get_nc()
    trace = bool(os.environ.get("KBENCH_TRACE"))
    res = run_bass_kernel_spmd(nc, in_maps, core_ids=list(range(N_CORES)), trace=trace)
    LAST_RESULTS = res
    outs = [np.asarray(res.results[c]["out"], dtype=np.float32) * (1.0 / S_W)
            for c in range(N_CORES)]
    return np.concatenate(outs, axis=0).reshape(4, 2048, 2048)


# revision 72
# speedup vs baseline: 1.0142x; 1.0016x over previous
"""LoRA-MoE layer (base dense + top-2 routed rank-16 LoRA experts) on 8 TRN2 cores.

Data-parallel over tokens (8192 -> 1024/core), weights replicated, zero
collectives.  The dense base projection runs as fp8e4 DoubleRow matmuls
(K=256 packed per instruction, 2 output columns/cycle) with a hi/lo split:
    x@W ~= xh@Wh + xh@Wl + xl@Wh      xh = fp8(x), xl = fp8(x - xh)
W/A/R are pre-scaled on host (x512/x32/x32) so their uniform(+-1/sqrt(2048))
values escape fp8's subnormal range; the W-scale rides through PSUM (Bc
carries it too) and is divided out on the host after gather.  The xl
correction is dropped for kp>=4 (half the contraction): measured full-batch
error 1.755e-2 against the 2e-2 gate, for 25% fewer base matmuls.

PE-sequencer economics dominate instruction choice: every Ldweights costs
~105ns of PE SEQ while a matmult costs ~2ns, so matmuls are grouped
(ti, ob-pair)-major so runs of 4-6 share one stationary, and a post-pass
(_dedupe_ldweights) deletes the redundant weight loads the tile scheduler
emits 1:1.

Schedule: 8 k-pair chunks; PSUM holds only ~6 rotating accumulators, so
sweeps S0 (kp0), S1 (kp1), S2 (kp2+3) evacuate per-tile partials into an
SBUF f32 accumulator (ACT/DVE copies + DVE adds - GPSIMD cannot touch PSUM
on real HW; a minority of adds go ACT-copy + Pool SBUF-add), and S3 (kp4-7)
fuses the fp8-DR LoRA-B finisher and a staging add before each store.  Token
tile 7 skips the early sweeps entirely and runs whole-K per-ob groups at the
tail, storing each ob as soon as it finishes.  The DMA stream is ordered so
every sweep's W chunks land just ahead of their consumers (W0 halves split
finer for the prologue, W1 behind x1, W2/W3 braided into the x tail, W4-7
last); merged hi/lo tensors keep the 625ns-per-descriptor HWDGE issue rate
off the critical path.

Routing: per (ti, kp) three ap-8 DoubleRow matmuls (xh.Rh + xh.Rl + xl.Rh)
ride the base x stationaries, accumulating token-major [128t, 8e] regions of
one PSUM bank as sequential per-ti bursts; the top-2 softmax chains (DVE/ACT)
consume them directly - no logit transposes.  u = A.x is 1-term fp8 DR.
Per-token weights are transposed and expanded to [er, t] via the one-hot Mm
matmul; us = u * wb is then re-packed to the DoubleRow [64, 2, t] layout with
PE selector matmuls (DVE cannot shift partitions) so the finisher runs fp8.
"""

import os
import sys

import numpy as np


def _ensure_concourse():
    try:
        import concourse  # noqa: F401
    except ImportError:
        for p in ("/opt/trn_rl_repo", os.path.expanduser("~/.axon_site/_ro/trn_rl_repo")):
            if os.path.isdir(p):
                sys.path.insert(0, p)
                break


_ensure_concourse()

import ml_dtypes  # noqa: E402
import concourse.bass as bass  # noqa: E402,F401
import concourse.tile as tile  # noqa: E402
from concourse import bacc, mybir  # noqa: E402

F32 = mybir.dt.float32
BF16 = mybir.dt.bfloat16
F8 = mybir.dt.float8e4
X_AX = mybir.AxisListType.X
ALU = mybir.AluOpType
ACT = mybir.ActivationFunctionType
DR = mybir.MatmulPerfMode.DoubleRow

N_CORES = 8
N_TOK = 8192          # total tokens (4 x 2048)
NT = N_TOK // N_CORES  # tokens per core = 1024
D = 2048
O = 2048
E = 8
R = 16
ER = E * R            # 128
KP = D // 256         # 8 k-pair chunks (256 contraction each, DoubleRow-packed)
TI = NT // 128        # 8 token tiles
OBS = 4               # o blocks of 512

S_W = 512.0           # host scale on W (and Bc); divided out on host
S_A = 32.0            # host scale on lora_A; divided out at u evac
S_R = 32.0            # host scale on router_w; divided out at chain head

_NC_CACHE = {}
LAST_RESULTS = None


def _emit_chain(nc, smallp, lg_reg, w_tiles):
    """Top-2 softmax weight chain for one 128-token tile (DVE/ACT ops).

    lg_reg is the token-major [128, 8] f32 PSUM region holding S_R * logits."""
    L = smallp.tile([128, E], F32, name="L", tag="L")
    nc.scalar.mul(L[:], lg_reg, 1.0 / S_R)
    m1 = smallp.tile([128, 1], F32, name="m1", tag="m1")
    nc.vector.reduce_max(m1[:], L[:], axis=X_AX)
    nm1 = smallp.tile([128, 1], F32, name="nm1", tag="nm1")
    nc.scalar.mul(nm1[:], m1[:], -1.0)
    # mask out the top-1 entry, then find the 2nd max
    msk = smallp.tile([128, E], F32, name="msk", tag="msk")
    nc.vector.tensor_scalar(msk[:], L[:], m1[:], -1e30, ALU.is_equal, ALU.mult)
    L2 = smallp.tile([128, E], F32, name="L2", tag="L2")
    nc.vector.tensor_tensor(L2[:], L[:], msk[:], ALU.add)
    m2 = smallp.tile([128, 1], F32, name="m2", tag="m2")
    nc.vector.reduce_max(m2[:], L2[:], axis=X_AX)
    eL = smallp.tile([128, E], F32, name="eL", tag="eL")
    nc.scalar.activation(eL[:], L[:], ACT.Exp, bias=nm1[:])
    ge = smallp.tile([128, E], F32, name="ge", tag="ge")
    nc.vector.tensor_scalar(ge[:], L[:], m2[:], None, ALU.is_ge)
    un = smallp.tile([128, E], F32, name="un", tag="un")
    nc.vector.tensor_tensor(un[:], eL[:], ge[:], ALU.mult)
    s = smallp.tile([128, 1], F32, name="s", tag="s")
    nc.vector.reduce_sum(s[:], un[:], axis=X_AX)
    r = smallp.tile([128, 1], F32, name="r", tag="r")
    nc.vector.reciprocal(r[:], s[:])
    r2 = smallp.tile([128, 1], F32, name="r2", tag="r2")
    nc.scalar.mul(r2[:], r[:], 2.0)  # fold SCALING = 2.0
    w = smallp.tile([128, E], BF16, name="w", tag="w", bufs=8)
    nc.vector.tensor_scalar(w[:], un[:], r2[:], None, ALU.mult)
    w_tiles.append(w)


def _body(tc, nc, xp, Wp, Ah, Rr, Bc, Sel, Mm, Idb, out):
    with (
        tc.tile_pool(name="const", bufs=1) as constp,
        tc.tile_pool(name="small", bufs=4) as smallp,
        tc.tile_pool(name="stage", bufs=4) as stagep,
    ):
        # PSUM: u(2) + lg(1) + rotation mm0..3 (4) + trwb (1) = 8 banks.
        ps_u = tc.alloc_tile_pool(name="ps_u", bufs=1, space="PSUM")
        ps_lg = tc.alloc_tile_pool(name="ps_lg", bufs=1, space="PSUM")
        ps_tr = tc.alloc_tile_pool(name="ps_tr", bufs=1, space="PSUM")
        ps_mm = tc.alloc_tile_pool(name="ps_mm", bufs=1, space="PSUM")

        # ---- resident SBUF tensors ----
        xp_sb = constp.tile([128, KP, 2, 2 * NT], F8, name="xp_sb")
        Wp_sb = constp.tile([128, KP, 2, 2 * O], F8, name="Wp_sb")
        Ah_sb = constp.tile([128, KP, 2, ER], F8, name="Ah_sb")
        Rr_sb = constp.tile([128, KP, 2, 32], F8, name="Rr_sb")
        Bc_sb = constp.tile([64, 2, O], F8, name="Bc_sb")
        Sel_sb = constp.tile([ER, ER], BF16, name="Sel_sb")
        us8_sb = constp.tile([64, 2, NT], F8, name="us8_sb")
        Mm_sb = constp.tile([E, ER], BF16, name="Mm_sb")
        Idb_sb = constp.tile([128, 128], BF16, name="Idb_sb")
        u_sb = constp.tile([ER, NT], F32, name="u_sb")
        us_sb = constp.tile([ER, NT], BF16, name="us_sb")
        wT_sb = constp.tile([E, NT], BF16, name="wT_sb")
        part_sb = constp.tile([128, 32 * 512], F32, name="part_sb")

        tiles = [(ti, ob) for ti in range(TI) for ob in range(OBS)]  # 32

        # PE p-state warm-up: the cost model runs PE at 0.65-1.2GHz for the
        # first 3us of busy time.  Burn the ramp on throwaway fp32 matmuls
        # (iota-seeded, no DMA dependency) while the first chunks stream in.
        # PE p-state warm-up: the model halves PE speed for the first 3us
        # of busy time; burn most of the ramp on throwaway matmuls while the
        # first DMA chunks land (out partitions = wu free dim = 8).
        wu_sb = constp.tile([128, 520], F32, name="wu_sb")
        wu_ps = ps_lg.tile([8, 512], F32, name="wu_ps", tag="lg",
                           padded_shape=[128, 512])
        nc.gpsimd.memset(wu_sb[:], 1.0)
        nc.tensor.matmul(wu_ps[:], wu_sb[:, 0:8], wu_sb[:, 8:520],
                         start=True, stop=True)

        # ---- DMA emission (single in-order SP/HWDGE stream) ----
        # One dma_start per merged hi/lo chunk (the 625ns HWDGE issue cost
        # otherwise rate-limits).  First x/W chunks split finer so PE starts
        # ~3us in; W kp1 behind x kp1 (S0/S1 never W-gated); W2 early, then
        # the x tail, W3 just ahead of S2's kp3 rows, Bc, and the W tail
        # ahead of S3.
        # first chunks issue from the ACT queue in parallel with SP's so
        # both DGE pipelines fill while the DMA engines are still empty
        nc.sync.dma_start(Ah_sb[:], Ah[:])
        nc.sync.dma_start(xp_sb[:, 0, :, 0:512], xp[:, 0, :, 0:512])
        nc.sync.dma_start(xp_sb[:, 0, :, 512:1024], xp[:, 0, :, 512:1024])
        nc.scalar.dma_start(Wp_sb[:, 0, :, 0:1024], Wp[:, 0, :, 0:1024])
        nc.sync.dma_start(xp_sb[:, 0, :, 1024:2048], xp[:, 0, :, 1024:2048])
        nc.sync.dma_start(Wp_sb[:, 0, :, 2048:3072], Wp[:, 0, :, 2048:3072])
        nc.sync.dma_start(Rr_sb[:], Rr[:])
        nc.sync.dma_start(Wp_sb[:, 0, :, 1024:2048], Wp[:, 0, :, 1024:2048])
        nc.sync.dma_start(Wp_sb[:, 0, :, 3072:4096], Wp[:, 0, :, 3072:4096])
        nc.sync.dma_start(Mm_sb[:], Mm[:])
        nc.sync.dma_start(Idb_sb[:], Idb[:])
        nc.sync.dma_start(xp_sb[:, 1], xp[:, 1])
        nc.sync.dma_start(Wp_sb[:, 1], Wp[:, 1])
        # x hi-halves first (base sweeps + u read only xh); the lo (xl)
        # halves of kp>=4 feed nothing but the S2 logit bursts, so they ship
        # after W3 - pulling every W chunk ~3us earlier
        for kp in range(2, 6):
            nc.sync.dma_start(xp_sb[:, kp, :, 0:NT], xp[:, kp, :, 0:NT])
        nc.sync.dma_start(xp_sb[:, 6, :, 0:NT], xp[:, 6, :, 0:NT])
        nc.sync.dma_start(Wp_sb[:, 2, :, 0:O], Wp[:, 2, :, 0:O])
        nc.sync.dma_start(xp_sb[:, 7, :, 0:NT], xp[:, 7, :, 0:NT])
        nc.sync.dma_start(Wp_sb[:, 2, :, O:2 * O], Wp[:, 2, :, O:2 * O])
        nc.sync.dma_start(Wp_sb[:, 3, :, 0:O], Wp[:, 3, :, 0:O])
        nc.sync.dma_start(xp_sb[:, 2, :, NT:2 * NT], xp[:, 2, :, NT:2 * NT])
        nc.sync.dma_start(xp_sb[:, 3, :, NT:2 * NT], xp[:, 3, :, NT:2 * NT])
        nc.sync.dma_start(Wp_sb[:, 3, :, O:2 * O], Wp[:, 3, :, O:2 * O])
        for kp in range(4, KP):
            nc.sync.dma_start(xp_sb[:, kp, :, NT:2 * NT],
                              xp[:, kp, :, NT:2 * NT])
        nc.sync.dma_start(Bc_sb[:], Bc[:])
        nc.sync.dma_start(Sel_sb[:], Sel[:])
        for kp in range(4, KP):
            nc.sync.dma_start(Wp_sb[:, kp], Wp[:, kp])

        # ---- u / lg accumulators ----
        u_ps = [ps_u.tile([ER, 512], F32, name=f"ups{tb}", tag=f"u{tb}")
                for tb in range(2)]
        lg_ps = None  # created at S2 (the lg bank joins the S0/S1 rotation)

        def xs(which, kp, sl):
            off = 0 if which == "h" else NT
            return xp_sb[:, kp, :, off + sl.start:off + sl.stop]

        def emit_ulg(kp):
            st, sp = (kp == 0), (kp == KP - 1)
            for tb in range(2):
                nc.tensor.matmul(u_ps[tb][:], Ah_sb[:, kp],
                                 xs("h", kp, slice(tb * 512, (tb + 1) * 512)),
                                 start=st, stop=sp, perf_mode=DR)

        lg_holder = []

        def emit_lg_burst(ti, kp_lo=0, kp_hi=KP):
            # one sequential accumulation group per [128t, 8e] region (the
            # interp allows a single pending group per psum tile): all 8 kp
            # x 3 hi/lo terms back-to-back, then the chain consumes it.
            reg = lg_holder[0][:, ti * E:(ti + 1) * E]
            tsl = slice(ti * 128, (ti + 1) * 128)
            for kp in range(kp_lo, kp_hi):
                for t_i, (which, roff) in enumerate(
                        (("h", 0), ("h", 16), ("l", 0))):
                    nc.tensor.matmul(reg, xs(which, kp, tsl),
                                     Rr_sb[:, kp, :, roff:roff + E],
                                     start=(kp == 0 and t_i == 0),
                                     stop=(kp == KP - 1 and t_i == 2),
                                     perf_mode=DR)
            if kp_hi == KP:
                _emit_chain(nc, smallp, reg, w_tiles)

        rot = [0]
        TAGS5 = [(ps_mm, "mm0"), (ps_mm, "mm1"), (ps_mm, "mm2"),
                 (ps_mm, "mm3"), (ps_tr, "trwb"), (ps_lg, "lg")]
        TAGS6 = [(ps_mm, "mm0"), (ps_mm, "mm1"), (ps_mm, "mm2"),
                 (ps_mm, "mm3"), (ps_u, "u0"), (ps_u, "u1"),
                 (ps_tr, "trwb")]
        TAGS7 = TAGS6 + [(ps_lg, "lg")]
        TAGS8 = TAGS7 + [(ps_tr, "trwb")]

        def rot_tile(name, tags):
            pool, tag = tags[rot[0] % len(tags)]
            rot[0] += 1
            return pool.tile([128, 512], F32, name=name, tag=tag, bufs=1)

        def emit_pair_group(ti, p, kps, tags, sweep, last_sweep=False):
            """One (token-tile, ob-pair) group: per kp, 4 xh-stationary
            matmuls (Wh/Wl x 2 ob) then 2 xl-stationary (Wh x 2 ob); the
            ldweights dedupe collapses each run to one weight load.  Returns
            the 2 psum tiles (ob 2p, 2p+1)."""
            tsl = slice(ti * 128, (ti + 1) * 128)
            pss = [rot_tile(f"{sweep}_{ti}_{2 * p + i}", tags)
                   for i in range(2)]
            first_kp, last_kp = kps[0], kps[-1]
            for kp in kps:
                for i in range(2):
                    ob = 2 * p + i
                    osl = slice(ob * 512, (ob + 1) * 512)
                    nc.tensor.matmul(pss[i][:], xs("h", kp, tsl),
                                     Wp_sb[:, kp, :, osl.start:osl.stop],
                                     start=(kp == first_kp), stop=False,
                                     perf_mode=DR)
                    nc.tensor.matmul(pss[i][:], xs("h", kp, tsl),
                                     Wp_sb[:, kp, :, O + osl.start:O + osl.stop],
                                     start=False, stop=False, perf_mode=DR)
                for i in range(2):
                    ob = 2 * p + i
                    osl = slice(ob * 512, (ob + 1) * 512)
                    # the xl correction is dropped for kp 6-7: the remaining
                    # x-quantization noise on 2/8 of the contraction is
                    # ~1.3% of the output against the 2e-2 gate, and it saves
                    # 2 matmuls per (pair, kp) plus the xl bytes of the
                    # stream tail
                    if kp >= 4:
                        continue
                    nc.tensor.matmul(pss[i][:], xs("l", kp, tsl),
                                     Wp_sb[:, kp, :, osl.start:osl.stop],
                                     start=False,
                                     stop=(kp == last_kp and not last_sweep),
                                     perf_mode=DR)
            return pss

        ev = [0]
        EV_COPY = ("act", "vec", "pool", "vec")
        EV_ADD = ("vec", "pool", "vec", "vec")

        def emit_evac(ps, idx, first):
            # kp0 sweep: copy psum -> f32 partial; later sweeps: partial +=
            # psum.  ACT only ever sees copies (no tensor_tensor on the
            # scalar engine); Pool's f32 tensor ops are ~2.3x slower than
            # DVE so it takes a minority share.
            # GPSIMD/Pool cannot access PSUM on real HW (BIR verifier), so
            # copies alternate ACT/DVE and adds are DVE-only.
            dst = part_sb[:, idx * 512:(idx + 1) * 512]
            e = (EV_COPY if first else EV_ADD)[ev[0] % 4]
            ev[0] += 1
            if first:
                if e in ("act", "pool"):
                    nc.scalar.copy(dst, ps[:])
                else:
                    nc.vector.tensor_copy(dst, ps[:])
            elif e == "pool":
                # relieve DVE: ACT evacuates PSUM to a scratch tile, Pool
                # (SBUF-only) folds it into the partial
                sc = stagep.tile([128, 512], F32, name="sc", tag="sc", bufs=2)
                nc.scalar.copy(sc[:], ps[:])
                nc.gpsimd.tensor_tensor(dst, dst, sc[:], ALU.add)
            else:
                nc.vector.tensor_tensor(dst, dst, ps[:], ALU.add)

        w_tiles = []

        def emit_wexpand(tb):
            for ti in range(tb * 4, tb * 4 + 4):
                sl = slice(ti * 128, (ti + 1) * 128)
                trW = ps_tr.tile([E, 128], BF16, name="trW", tag="trwb",
                                 padded_shape=[128, 1024])
                nc.tensor.transpose(trW[:], w_tiles[ti][:], Idb_sb[:])
                nc.scalar.copy(wT_sb[:, sl], trW[:])
            sl = slice(tb * 512, (tb + 1) * 512)
            wb_ps = ps_tr.tile([ER, 512], F32, name="wbps", tag="trwb")
            nc.tensor.matmul(wb_ps[:], Mm_sb[:], wT_sb[:, sl],
                             start=True, stop=True)
            nc.vector.tensor_tensor(us_sb[:, sl], u_sb[:, sl],
                                    wb_ps[:], ALU.mult)
            # pack us rows into the DoubleRow [64, 2, t] layout for the fp8
            # finisher: PE selector matmuls move er 64..127 onto partitions
            # 0..63 (DVE cannot shift partitions), ACT casts psum -> fp8
            for j in range(2):
                pk = ps_tr.tile([64, 512], F32, name="pk", tag="trwb",
                                padded_shape=[128, 512])
                nc.tensor.matmul(pk[:], Sel_sb[:, j * 64:(j + 1) * 64],
                                 us_sb[:, sl], start=True, stop=True)
                if j == 0:
                    nc.scalar.copy(us8_sb[:, j, sl.start:sl.stop], pk[:])
                else:
                    nc.vector.tensor_copy(us8_sb[:, j, sl.start:sl.stop],
                                          pk[:])

        def emit_finish(pss4, ti, copy_stage=False):
            # fused bf16 LoRA-B finishers for all 4 ob tiles of this token
            # tile (one shared us stationary), then staging (+ partial when
            # the tile ran the early sweeps) and the store.
            tsl = slice(ti * 128, (ti + 1) * 128)
            for ob in range(OBS):
                nc.tensor.matmul(pss4[ob][:], us8_sb[:, :, tsl.start:tsl.stop],
                                 Bc_sb[:, :, ob * 512:(ob + 1) * 512],
                                 start=False, stop=True, perf_mode=DR)
            for ob in range(OBS):
                idx = ti * OBS + ob
                st = stagep.tile([128, 512], BF16, name="st", tag="st", bufs=6)
                if copy_stage:
                    if ob % 2 == 0:
                        nc.scalar.copy(st[:], pss4[ob][:])
                    else:
                        nc.vector.tensor_copy(st[:], pss4[ob][:])
                else:
                    nc.vector.tensor_tensor(st[:], pss4[ob][:],
                                            part_sb[:, idx * 512:(idx + 1) * 512],
                                            ALU.add)
                nc.sync.dma_start(out[tsl, ob * 512:(ob + 1) * 512], st[:])

        NTI = TI - 1  # ti7 skips the early sweeps: full sweep at the tail

        # ---- S0: kp0 sweep (ti 0..6); u(0) up front, u(1..2) as x lands.
        # p-major order: all ob-pair-0 groups need only the first halves of
        # the W0 stream, so PE stops chasing the W0 DMA after ~2 chunks. ----
        emit_ulg(0)

        def s0_hi(ti, pss):
            # all rows reading the Wh half of W0 (xh.Wh and xl.Wh) so the
            # prologue isn't gated on the later Wl sub-chunk
            tsl = slice(ti * 128, (ti + 1) * 128)
            for which in ("h", "l"):
                for i in range(2):
                    osl = slice(i * 512, (i + 1) * 512)
                    nc.tensor.matmul(pss[i][:], xs(which, 0, tsl),
                                     Wp_sb[:, 0, :, osl.start:osl.stop],
                                     start=(which == "h"), stop=False,
                                     perf_mode=DR)

        def s0_lo(ti, pss):
            tsl = slice(ti * 128, (ti + 1) * 128)
            for i in range(2):
                osl = slice(i * 512, (i + 1) * 512)
                nc.tensor.matmul(pss[i][:], xs("h", 0, tsl),
                                 Wp_sb[:, 0, :, O + osl.start:O + osl.stop],
                                 start=False, stop=True, perf_mode=DR)

        g0 = [rot_tile("s0e_0_%d" % i, TAGS5) for i in range(2)]
        g1 = [rot_tile("s0e_1_%d" % i, TAGS5) for i in range(2)]
        s0_hi(0, g0)
        s0_hi(1, g1)
        s0_lo(0, g0)
        for i in range(2):
            emit_evac(g0[i], 0 * OBS + i, first=True)
        g2 = [rot_tile("s0e_2_%d" % i, TAGS5) for i in range(2)]
        s0_hi(2, g2)
        s0_lo(1, g1)
        for i in range(2):
            emit_evac(g1[i], 1 * OBS + i, first=True)
        s0_lo(2, g2)
        for i in range(2):
            emit_evac(g2[i], 2 * OBS + i, first=True)
        for p in range(2):
            for ti in range(3 if p == 0 else 0, NTI):
                if (ti, p) == (0, 1):
                    emit_ulg(1)
                if (ti, p) == (4, 1):
                    emit_ulg(2)
                pss = emit_pair_group(ti, p, [0], TAGS5, "s0")
                for i in range(2):
                    emit_evac(pss[i], ti * OBS + 2 * p + i, first=True)

        # ---- S1: kp1 sweep; u(3..7) braided by x arrival; u evac ----
        emit_ulg(3)
        for ti in range(NTI):
            for p in range(2):
                if (ti, p) == (1, 1):
                    emit_ulg(4)
                if (ti, p) == (3, 1):
                    emit_ulg(5)
                if (ti, p) == (5, 1):
                    emit_ulg(6)
                pss = emit_pair_group(ti, p, [1], TAGS5, "s1")
                for i in range(2):
                    emit_evac(pss[i], ti * OBS + 2 * p + i, first=False)
        emit_ulg(7)
        for tb in range(2):
            nc.scalar.mul(u_sb[:, tb * 512:(tb + 1) * 512], u_ps[tb][:],
                          1.0 / S_A)

        # ---- S2: kp2+3 paired sweep; lg bursts + chains; w expansions ----
        lg_holder.append(ps_lg.tile([128, 8 * E], F32, name="lg", tag="lg",
                                    padded_shape=[128, 512]))
        quarters = [(b, q) for b in range(TI) for q in range(4)]
        hslot = [0]

        def emit_quarter():
            if hslot[0] < len(quarters):
                b, q = quarters[hslot[0]]
                hslot[0] += 1
                emit_lg_burst(b, 2 * q, 2 * q + 2)

        for ti in range(NTI):
            for p in range(2):
                # quarter-bursts (4 Ldweights ~0.42us of PE SEQ each)
                # braided 2-3 per group slot: each slot's sequencer load
                # stays under the group's 1.28us engine shadow
                if (ti, p) >= (0, 1):
                    emit_quarter()
                    emit_quarter()
                if (ti, p) == (4, 1):
                    emit_wexpand(0)
                pss = emit_pair_group(ti, p, [2, 3], TAGS6, "s2")
                for i in range(2):
                    emit_evac(pss[i], ti * OBS + 2 * p + i, first=False)

        # ---- S3: ti7's first two whole-K ob groups lead (their kp0-3 rows
        # are DMA-independent, absorbing the W tail window), then the per-ti
        # kp4..7 sweeps + finishers, then ti7's deferred finishers and its
        # last two obs.  The last stores issue from the ACT/DVE queues so
        # their HWDGE work runs off the SP path in the kernel tail. ----
        tsl7 = slice(7 * 128, 8 * 128)

        def ti7_base(ob, ps):
            osl = slice(ob * 512, (ob + 1) * 512)
            for kp in range(KP):
                nc.tensor.matmul(ps[:], xs("h", kp, tsl7),
                                 Wp_sb[:, kp, :, osl.start:osl.stop],
                                 start=(kp == 0), stop=False, perf_mode=DR)
                nc.tensor.matmul(ps[:], xs("h", kp, tsl7),
                                 Wp_sb[:, kp, :, O + osl.start:O + osl.stop],
                                 start=False, stop=False, perf_mode=DR)
                if kp < 4:
                    nc.tensor.matmul(ps[:], xs("l", kp, tsl7),
                                     Wp_sb[:, kp, :, osl.start:osl.stop],
                                     start=False, stop=False, perf_mode=DR)

        def ti7_fin(ob, ps, eng):
            osl = slice(ob * 512, (ob + 1) * 512)
            nc.tensor.matmul(ps[:], us8_sb[:, :, tsl7.start:tsl7.stop],
                             Bc_sb[:, :, osl.start:osl.stop],
                             start=False, stop=True, perf_mode=DR)
            st = stagep.tile([128, 512], BF16, name="st", tag="st", bufs=6)
            if eng == "act":
                nc.scalar.copy(st[:], ps[:])
            else:
                nc.vector.tensor_copy(st[:], ps[:])
            nc.sync.dma_start(out[tsl7, osl], st[:])

        TAGS6C = [(ps_mm, "mm0"), (ps_mm, "mm1"), (ps_mm, "mm2"),
                  (ps_mm, "mm3"), (ps_lg, "lg"), (ps_tr, "trwb")]
        # ti7 ob0/ob1 park on the freed u banks (outside the rotation) so
        # their finishers can wait for us8 without blocking the loop
        t7a = ps_u.tile([128, 512], F32, name="s3f_0", tag="u0", bufs=1)
        emit_quarter()
        emit_quarter()
        ti7_base(0, t7a)
        emit_quarter()
        emit_quarter()
        t7b = ps_u.tile([128, 512], F32, name="s3f_1", tag="u1", bufs=1)
        ti7_base(1, t7b)
        while hslot[0] < len(quarters):
            emit_quarter()
        emit_wexpand(1)
        for ti in range(NTI):
            pssA = emit_pair_group(ti, 0, [4, 5, 6, 7], TAGS6C, "s3",
                                   last_sweep=True)
            pssB = emit_pair_group(ti, 1, [4, 5, 6, 7], TAGS6C, "s3",
                                   last_sweep=True)
            emit_finish(pssA + pssB, ti)
        ti7_fin(0, t7a, "act")
        ti7_fin(1, t7b, "vec")
        t7c = rot_tile("s3f_2", TAGS6C)
        ti7_base(2, t7c)
        ti7_fin(2, t7c, "act")
        t7d = rot_tile("s3f_3", TAGS6C)
        ti7_base(3, t7d)
        ti7_fin(3, t7d, "vec")

        ps_mm.release()
        ps_tr.release()
        ps_lg.release()
        ps_u.release()


def _ldweights_key(inst):
    ap = inst.ins[0]
    return (str(ap), str(inst.perf_mode), str(inst.is_transpose),
            str(inst.tile_position), str(inst.tile_size))


def _dedupe_ldweights(nc):
    """Drop an InstLdweights when the PE array already holds the same
    stationary (identical weights AP, only paired matmults in between).
    The ~105ns-per-instruction PE sequencer cost of redundant weight loads
    otherwise dominates the kernel."""
    removed = 0
    for bb in nc.m.functions[0].blocks:
        keep = []
        last_key = None
        for inst in bb.instructions:
            t = type(inst).__name__
            if t == "InstLdweights":
                k = _ldweights_key(inst)
                si = inst.sync_info
                has_sync = si is not None and (list(si.on_wait) or
                                               list(si.on_update))
                if k == last_key and not has_sync:
                    removed += 1
                    continue
                last_key = k
            elif t != "InstMatmult":
                if getattr(inst, "engine", None) == mybir.EngineType.PE:
                    last_key = None
            keep.append(inst)
        bb.instructions = keep
    return removed


def build_nc():
    nc = bacc.Bacc("TRN2", target_bir_lowering=False, debug=False, num_devices=N_CORES)
    xp = nc.dram_tensor("xp", [128, KP, 2, 2 * NT], F8, kind="ExternalInput").ap()
    Wp = nc.dram_tensor("Wp", [128, KP, 2, 2 * O], F8, kind="ExternalInput").ap()
    Ah = nc.dram_tensor("Ah", [128, KP, 2, ER], F8, kind="ExternalInput").ap()
    Rr = nc.dram_tensor("Rr", [128, KP, 2, 32], F8, kind="ExternalInput").ap()
    Bc = nc.dram_tensor("Bc", [64, 2, O], F8, kind="ExternalInput").ap()
    Sel = nc.dram_tensor("Sel", [ER, ER], BF16, kind="ExternalInput").ap()
    Mm = nc.dram_tensor("Mm", [E, ER], BF16, kind="ExternalInput").ap()
    Idb = nc.dram_tensor("Idb", [128, 128], BF16, kind="ExternalInput").ap()
    out = nc.dram_tensor("out", [NT, O], BF16, kind="ExternalOutput").ap()
    with tile.TileContext(nc) as tc:
        _body(tc, nc, xp, Wp, Ah, Rr, Bc, Sel, Mm, Idb, out)
    _dedupe_ldweights(nc)
    nc.compile()
    return nc


def get_nc():
    if "nc" not in _NC_CACHE:
        _NC_CACHE["nc"] = build_nc()
    return _NC_CACHE["nc"]


F8NP = ml_dtypes.float8_e4m3


def _pack_k(aT):
    """[D, C] -> [128, KP, 2, C]: element [p, kp, j, :] holds row k=kp*256+j*128+p."""
    C = aT.shape[1]
    return np.ascontiguousarray(
        aT.reshape(KP, 2, 128, C).transpose(2, 0, 1, 3))


def _hi_lo(aT):
    hi = aT.astype(F8NP)
    lo = (aT - hi.astype(np.float32)).astype(F8NP)
    return hi, lo


def make_in_maps(x, weight, lora_A, lora_B, router_w):
    x = np.ascontiguousarray(np.asarray(x, dtype=np.float32)).reshape(N_TOK, D)
    weight = np.asarray(weight, dtype=np.float32)
    lora_A = np.asarray(lora_A, dtype=np.float32)
    lora_B = np.asarray(lora_B, dtype=np.float32)
    router_w = np.asarray(router_w, dtype=np.float32)

    WTh, WTl = _hi_lo(np.ascontiguousarray(weight.T) * S_W)
    Wpm = np.concatenate([_pack_k(WTh), _pack_k(WTl)], axis=3)
    ATh = (np.ascontiguousarray(lora_A.reshape(ER, D).T) * S_A).astype(F8NP)
    Ahm = _pack_k(ATh)
    RT = np.zeros((D, 16), dtype=np.float32)
    RT[:, :E] = router_w.T * S_R
    RTh, RTl = _hi_lo(RT)
    Rrm = np.concatenate([_pack_k(RTh), _pack_k(RTl)], axis=3)
    BcT = lora_B.transpose(0, 2, 1).reshape(ER, O) * S_W
    # DR-packed: Bc8[p, j, o] = BcT[j*64 + p, o]
    Bcm = np.ascontiguousarray(BcT.reshape(2, 64, O).transpose(1, 0, 2)).astype(F8NP)
    Selm = np.zeros((ER, ER), dtype=np.float32)
    for j in range(2):
        for m in range(64):
            Selm[j * 64 + m, j * 64 + m] = 1.0
    # lhsT selector: out[m,t] = sum_er Sel[er, j*64+m-block] us[er, t]
    Selm = Selm.astype(ml_dtypes.bfloat16)
    Mmm = np.zeros((E, ER), dtype=np.float32)
    for e in range(E):
        Mmm[e, e * R:(e + 1) * R] = 1.0
    Mmm = Mmm.astype(ml_dtypes.bfloat16)
    Idb = np.eye(128, dtype=np.float32).astype(ml_dtypes.bfloat16)

    in_maps = []
    for c in range(N_CORES):
        xT = np.ascontiguousarray(x[c * NT:(c + 1) * NT].T)
        xTh, xTl = _hi_lo(xT)
        in_maps.append({
            "xp": np.concatenate([_pack_k(xTh), _pack_k(xTl)], axis=3),
            "Wp": Wpm,
            "Ah": Ahm,
            "Rr": Rrm,
            "Bc": Bcm,
            "Sel": Selm,
            "Mm": Mmm,
            "Idb": Idb,
        })
    return in_maps


def kernel(x, weight, lora_A, lora_B, router_w):
    global LAST_RESULTS
    from concourse.bass_utils import run_bass_kernel_spmd

    in_maps = make_in_maps(x, weight, lora_A, lora_B, router_w)
    nc = get_nc()
    trace = bool(os.environ.get("KBENCH_TRACE"))
    res = run_bass_kernel_spmd(nc, in_maps, core_ids=list(range(N_CORES)), trace=trace)
    LAST_RESULTS = res
    outs = [np.asarray(res.results[c]["out"], dtype=np.float32) * (1.0 / S_W)
            for c in range(N_CORES)]
    return np.concatenate(outs, axis=0).reshape(4, 2048, 2048)
